# revision 3
# baseline (speedup 1.0000x reference)
"""Trainium2 Bass kernel for a CapsuleNet dynamic-routing layer (v2).

Math (per batch element b):
    u_hat[b,i,o,d] = sum_k W[i,o,d,k] * x[b,i,k]      # B=256, IC=1152, OC=10, OD=16, ID=8
    b_log = 0
    for it in 0..2:
        c = softmax(b_log, axis=o)
        s[b,o,d] = sum_i c[b,i,o] * u_hat[b,i,o,d]
        v = squash(s)
        if it < 2: b_log += sum_d u_hat * v

Sharding: data-parallel over B across 8 cores (32 local rows), W replicated.

v2 changes vs v1 (cost-model driven; ~272us -> ~223us modeled):
  - W loaded once (chunked HWDGE DMAs, xt first) instead of 16 Pool-queue
    DMAs; iter-0 s = 0.1*sum_i u_hat is ONE accumulating 72-matmul GEMM
    (M=32) instead of 288 M=8 matmuls, with a single batched squash.
  - c block-diag built by a masked broadcast-multiply on DVE (2x mode)
    instead of 64 Pool-queue scatter DMAs (994ns prep each); the mask
    writes the zeros so no memsets/persistent buffers are needed.
  - blg/c stored o-last so agreement writes and softmax are packed (2x).
  - PSUM drains split ACT/DVE only (GPSIMD cannot access PSUM on HW);
    agreement split DVE/Pool ~5:1 (DVE 2x = 0.52 ns/col vs Pool 1.98).
  - iter-0 agreement interleaved with the u_hat build per e-chunk; iters
    1-2 run softmax+cbt for all b-blocks first so the PE stage-1 stream
    overlaps the extraction/agreement wave.
  - one merged output DMA (SWDGE) via an [8, NBB, F] staging tile.
"""

import sys

sys.path.insert(0, "/opt/trn_rl_repo")

from contextlib import ExitStack

import numpy as np

import concourse.bass as bass
import concourse.tile as tile
from concourse import mybir

BF = mybir.dt.float16
F8 = mybir.dt.float8e4
F32 = mybir.dt.float32
import os
BD_DT = F8 if os.environ.get("BD_FP8", "0") == "1" else BF
AX = mybir.AxisListType
AF = mybir.ActivationFunctionType

N_CORES = 8
B_FULL, IC, OC, OD, ID = 256, 1152, 10, 16, 8
B_LOC = B_FULL // N_CORES          # 32
NIB = IC // 16                     # 72 i-blocks of 16
NBB = B_LOC // 8                   # 4 b-blocks of 8
F = OC * OD                        # 160

def _squash(nc, smp, ps, scale, vout):
    """vout = squash(scale * ps) with ps an [P, 160] psum slab (f32).

    squash(s) = (n^2/(1+n^2)) * s/(n + 1e-8),  n = ||s||_2 over d.
    The 1e-8 is dropped (n >= 1e-3 in routing; rel err < 1e-5).
    """
    P = ps.shape[0]
    sq = smp.tile([P, F], F32, tag="sq")
    nc.scalar.activation(sq[:], ps[:], AF.Square, scale=float(scale))
    n2 = smp.tile([P, OC], F32, tag="n2")
    nc.vector.tensor_reduce(
        n2[:], sq[:].rearrange("p (o d) -> p o d", d=OD), axis=AX.X,
        op=mybir.AluOpType.add)
    n1 = smp.tile([P, OC], F32, tag="n1")
    nc.vector.tensor_scalar_add(n1[:], n2[:], 1.0)
    sn = smp.tile([P, OC], F32, tag="sn")
    nc.scalar.sqrt(sn[:], n2[:])
    t1 = smp.tile([P, OC], F32, tag="t1")
    nc.vector.tensor_mul(t1[:], n1[:], sn[:])
    r1 = smp.tile([P, OC], F32, tag="r1")
    nc.vector.reciprocal(r1[:], t1[:])
    f1 = smp.tile([P, OC], F32, tag="f1")
    nc.vector.tensor_mul(f1[:], n2[:], r1[:])
    if scale != 1.0:
        nc.vector.tensor_scalar_mul(f1[:], f1[:], float(scale))
    nc.vector.tensor_mul(
        vout[:].rearrange("p (o d) -> p o d", d=OD),
        ps[:].rearrange("p (o d) -> p o d", d=OD),
        f1[:].unsqueeze(-1).broadcast_to((P, OC, OD)))


def _split_multiwait(nc):
    """Walrus encodes at most ONE semaphore wait per engine/DMA instruction.
    Hoist excess waits onto same-engine NoOps placed directly before the
    instruction.  HWDGE DMAs can't be gated that way - assert instead."""
    for fn in nc.m.functions:
        for bb in fn.blocks:
            out = []
            k = 0
            for ins in bb.instructions:
                si = ins.sync_info
                waits = list(si.on_wait) if si is not None and si.on_wait else []
                limit = 1
                if ins.opcode == "DMACopy":
                    q = str(getattr(ins, "queue", "") or "")
                    if "HW" in q and len(waits) > 1:
                        raise AssertionError(
                            f"HWDGE DMA {ins.name} has {len(waits)} waits: {ins}")
                if len(waits) > limit:
                    for w in waits[:-limit]:
                        nop = mybir.InstNoOp(name=f"{ins.name}-wn{k}", ins=[], outs=[])
                        k += 1
                        nop.engine = ins.engine
                        nop.sync_info = mybir.SyncInfo(on_wait=[w], on_update=[])
                        out.append(nop)
                    ins.sync_info = mybir.SyncInfo(
                        on_wait=waits[-limit:],
                        on_update=list(si.on_update) if si.on_update else [])
                out.append(ins)
            bb.instructions = out


def build_program(split_waits=True):
    nc = bass.Bass()
    bd_d = nc.declare_dram_parameter("bd", [8, 128, 9, NBB, 128], BD_DT, isOutput=False)
    xt_d = nc.declare_dram_parameter("xt", [128, NIB, B_LOC], BF, isOutput=False)
    ws_d = nc.declare_dram_parameter("ws", [128, NIB, F], BF, isOutput=False)
    msk_d = nc.declare_dram_parameter("msk", [80, F], BF, isOutput=False)
    o80_d = nc.declare_dram_parameter("o80", [80, 8], BF, isOutput=False)
    sel_d = nc.declare_dram_parameter("sel", [8, 128], BF, isOutput=False)
    selb_d = nc.declare_dram_parameter("selb", [B_LOC, NBB, 128], BF, isOutput=False)
    mk80_d = nc.declare_dram_parameter("mk80", [128, 80], BF, isOutput=False)
    out_d = nc.declare_dram_parameter("out", [B_LOC, F], F32, isOutput=True)

    with ExitStack() as ctx:
        tc = ctx.enter_context(tile.TileContext(nc))
        st = ctx.enter_context(tc.tile_pool(name="st", bufs=1))
        bdp = ctx.enter_context(tc.tile_pool(name="bdp", bufs=2))
        y2p = ctx.enter_context(tc.tile_pool(name="y2p", bufs=1))
        tsp = ctx.enter_context(tc.tile_pool(name="tsp", bufs=1))
        mkp = ctx.enter_context(tc.tile_pool(name="mkp", bufs=2))
        cnp = ctx.enter_context(tc.tile_pool(name="cnp", bufs=2))
        cbp = ctx.enter_context(tc.tile_pool(name="cbp", bufs=1))
        vxp = ctx.enter_context(tc.tile_pool(name="vxp", bufs=2))
        smp = ctx.enter_context(tc.tile_pool(name="smp", bufs=4))
        pbig = ctx.enter_context(tc.tile_pool(name="pbig", bufs=4, space="PSUM"))
        ps1p = ctx.enter_context(tc.tile_pool(name="ps1p", bufs=1, space="PSUM"))
        ps0p = ctx.enter_context(tc.tile_pool(name="ps0p", bufs=1, space="PSUM"))
        pvxp = ctx.enter_context(tc.tile_pool(name="pvxp", bufs=1, space="PSUM"))
        psvp = ctx.enter_context(tc.tile_pool(name="psvp", bufs=1, space="PSUM"))

        # --- persistent tiles ---
        u_hat = st.tile([128, NIB, NBB, F], BF, tag="u_hat")
        ws_sb = st.tile([128, NIB, F], BF, tag="ws_sb")
        xt_sb = st.tile([128, NIB, B_LOC], BF, tag="xt_sb")
        blg = st.tile([128, NBB, NIB, OC], BF, tag="blg")
        c_sb = st.tile([128, NBB, NIB, OC], BF, tag="c_sb")
        msk_sb = st.tile([80, F], BF, tag="msk_sb")
        o80_sb = st.tile([80, 8], BF, tag="o80_sb")
        sel_sb = st.tile([8, 128], BF, tag="sel_sb")
        selb_sb = st.tile([B_LOC, NBB, 128], BF, tag="selb_sb")
        mk80_sb = st.tile([128, 80], BF, tag="mk80_sb")
        v8 = [st.tile([8, F], BF, tag=f"v8_{i}", name=f"v8_{i}") for i in range(NBB)]
        vx4 = [st.tile([128, F], BF, tag=f"vx{i}", name=f"vx{i}") for i in range(NBB)]
        vall = st.tile([B_LOC, F], BF, tag="vall")
        og = st.tile([8, NBB, F], F32, tag="og")

        # --- input loads + zero-init of the c-blockdiag ---
        nc.sync.dma_start(out=xt_sb[:], in_=xt_d[:])
        for wc in range(4):
            nc.scalar.dma_start(
                out=ws_sb[:, wc * 18:(wc + 1) * 18, :],
                in_=ws_d[:, wc * 18:(wc + 1) * 18, :])
        nc.sync.dma_start(out=selb_sb[:], in_=selb_d[:])
        nc.sync.dma_start(out=sel_sb[:], in_=sel_d[:])
        nc.scalar.dma_start(out=msk_sb[:], in_=msk_d[:])
        nc.scalar.dma_start(out=o80_sb[:], in_=o80_d[:])
        nc.sync.dma_start(out=mk80_sb[:], in_=mk80_d[:])

        # --- iter-0 stage 1: s0 = 0.1*sum_i u_hat as one GEMM chain over the
        # full (i,k) contraction: xt[128,(iblk),32] x ws[128,(iblk),160] ---
        ps0 = ps0p.tile([B_LOC, F], F32, tag="ps0", name="ps0")
        for j in range(NIB):
            nc.tensor.matmul(
                ps0[:], lhsT=xt_sb[:, j, :], rhs=ws_sb[:, j, :],
                start=(j == 0), stop=(j == NIB - 1))

        def load_vx(bblk, first):
            pvx = pvxp.tile([128, F], F32, tag="pvx")
            if first:
                nc.tensor.matmul(
                    pvx[:], lhsT=selb_sb[:, bblk, :], rhs=vall[:],
                    start=True, stop=True)
            else:
                nc.tensor.matmul(
                    pvx[:], lhsT=sel_sb[:], rhs=v8[bblk][:],
                    start=True, stop=True)
            nc.scalar.copy(vx4[bblk][:], pvx[:])

        def agreement_slice(bblk, j0, j1, first, eng):
            """blg[:, bblk, j0:j1, :] (+)= sum_d u_hat * v for a j-slice."""
            w = j1 - j0
            pool = eng is nc.gpsimd
            y2 = y2p.tile([128, w, F], BF, tag="y2p3" if pool else "y2")
            eng.tensor_mul(
                y2[:], u_hat[:, j0:j1, bblk, :],
                vx4[bblk][:].unsqueeze(1).broadcast_to((128, w, F)))
            # in-place pairwise-add tree over d (fp16, 2x mode)
            y2v = y2[:].rearrange("p j (o d) -> p j o d", d=OD)
            eng.tensor_add(
                y2v[:, :, :, 0:8], y2v[:, :, :, 0:8], y2v[:, :, :, 8:16])
            eng.tensor_add(
                y2v[:, :, :, 0:4], y2v[:, :, :, 0:4], y2v[:, :, :, 4:8])
            eng.tensor_add(
                y2v[:, :, :, 0:2], y2v[:, :, :, 0:2], y2v[:, :, :, 2:4])
            dst = blg[:, bblk, j0:j1, :]
            if first:
                eng.tensor_add(dst, y2v[:, :, :, 0], y2v[:, :, :, 1])
            else:
                ts = tsp.tile([128, w, OC], BF, tag="tsp3" if pool else "ts")
                eng.tensor_add(ts[:], y2v[:, :, :, 0], y2v[:, :, :, 1])
                eng.tensor_add(dst, dst, ts[:])


        # --- u_hat build: one matmul per (iblk, bblk), K=128=(ii,k), N=160;
        # PSUM drained to SBUF across ACT/DVE/Pool.  Iter-0 squash runs first
        # so each e-chunk's agreement slices interleave with the build. ---
        _squash(nc, smp, ps0[:], 0.1, vall)
        for bblk in range(NBB):
            load_vx(bblk, first=True)
        drain_seq = [nc.scalar, nc.scalar, nc.scalar, nc.vector, nc.scalar,
                     nc.scalar, nc.vector, nc.scalar, nc.scalar, nc.vector]
        ndrain = 0
        nagr = 0
        for e in range(8):
            bdt = bdp.tile([128, 9, NBB, 128], BD_DT, tag="bdt")
            nc.gpsimd.dma_start(out=bdt[:], in_=bd_d[e])
            for j in range(9):
                iblk = e * 9 + j
                for h in range(2):
                    ps = pbig.tile([128, 2, F], F32, tag="pbig")
                    for bb in range(2):
                        nc.tensor.matmul(
                            ps[:, bb, :], lhsT=bdt[:, j, h * 2 + bb, :],
                            rhs=ws_sb[:, iblk, :], start=True, stop=True)
                    eng = drain_seq[ndrain % len(drain_seq)]
                    ndrain += 1
                    if eng is nc.scalar:
                        eng.copy(u_hat[:, iblk, h * 2:(h + 1) * 2, :], ps[:])
                    else:
                        eng.tensor_copy(u_hat[:, iblk, h * 2:(h + 1) * 2, :], ps[:])
            # iter-0 agreement for this e's 9 j-columns, all b-blocks
            for bblk in range(NBB):
                eng = nc.gpsimd if nagr % 4 == 3 else nc.vector
                nagr += 1
                agreement_slice(bblk, e * 9, e * 9 + 9, True, eng)

        # --- iters 1, 2: software-pipelined across bblks and iterations.
        # softmax+cbt feed the PE stage-1 stream; each bblk's iter-2 chain
        # starts as soon as its iter-1 agreement lands. ---
        def softmax_cbt(it, bblk):
            nc.scalar.activation(
                c_sb[:, bblk, :, :], blg[:, bblk, :, :], AF.Exp)
            sm = smp.tile([128, NIB], F32, tag="sm")
            nc.vector.tensor_reduce(
                sm[:], c_sb[:, bblk, :, :], axis=AX.X, op=mybir.AluOpType.add)
            rr = smp.tile([128, NIB], BF, tag="rr")
            with nc.allow_low_precision(reason="softmax denom recip in fp16"):
                nc.vector.reciprocal(rr[:], sm[:])
            cn = cnp.tile([128, NIB, OC], BF, tag="cn")
            nc.vector.tensor_mul(
                cn[:], c_sb[:, bblk, :, :],
                rr[:].unsqueeze(-1).broadcast_to((128, NIB, OC)))
            eng = nc.vector
            cbt = cbp.tile([128, NIB, 80], BF, tag=f"cbt{bblk % 2}")
            eng.tensor_mul(
                cbt[:],
                cn[:].unsqueeze(2).broadcast_to((128, NIB, 8, OC)),
                mk80_sb[:].rearrange("p (b o) -> p b o", o=OC).unsqueeze(1)
                .broadcast_to((128, NIB, 8, OC)))
            ps1 = ps1p.tile([80, F], F32, tag="ps1", name=f"ps1_{it}_{bblk}")
            for j in range(NIB):
                nc.tensor.matmul(
                    ps1[:], lhsT=cbt[:, j, :],
                    rhs=u_hat[:, j, bblk, :],
                    start=(j == 0), stop=(j == NIB - 1))
            return ps1

        def extract_s(ps1):
            mskd = mkp.tile([80, F], BF, tag="mskd")
            nc.vector.tensor_mul(mskd[:], ps1[:], msk_sb[:])
            psv = psvp.tile([8, F], F32, tag="psv")
            nc.tensor.matmul(
                psv[:], lhsT=o80_sb[:], rhs=mskd[:], start=True, stop=True)
            return psv

        for it in (1, 2):
            ps1s = [softmax_cbt(it, bb) for bb in range(NBB)]
            for bblk in range(NBB):
                psv = extract_s(ps1s[bblk])
                if it == 1:
                    _squash(nc, smp, psv[:], 1.0, v8[bblk])
                    load_vx(bblk, first=False)
                    agreement_slice(bblk, 0, 30, False, nc.vector)
                    agreement_slice(bblk, 30, 60, False, nc.vector)
                    agreement_slice(bblk, 60, 72, False, nc.gpsimd)
                else:
                    _squash(nc, smp, psv[:], 1.0, og[:, bblk, :])
        nc.gpsimd.dma_start(
            out=out_d[:].rearrange("(bb bp) f -> bp bb f", bp=8), in_=og[:])

    if split_waits:
        _split_multiwait(nc)
    return nc


def _host_inputs(x, W):
    """Per-core input maps from full x [256,1152,8] f32, W [1,1152,10,16,8] f32."""
    bf = np.float16
    f8 = mybir.dt.np(BD_DT)
    W0 = np.asarray(W[0], dtype=np.float32)
    # ws[q=(ii,k), j, (o,d)] = W[j*16+ii, o, d, k]
    ws = np.ascontiguousarray(
        W0.reshape(NIB, 16, OC, OD, ID).transpose(1, 4, 0, 2, 3)
        .reshape(128, NIB, F)).astype(bf)
    msk = np.zeros((80, F), dtype=bf)
    for bpp in range(8):
        for o in range(OC):
            msk[bpp * 10 + o, o * OD:(o + 1) * OD] = 1.0
    o80 = np.zeros((80, 8), dtype=bf)
    for p in range(80):
        o80[p, p // 10] = 1.0
    sel = np.zeros((8, 128), dtype=bf)
    for p in range(128):
        sel[p // 16, p] = 1.0
    mk80 = np.zeros((128, 80), dtype=bf)
    for p in range(128):
        mk80[p, (p // 16) * 10:(p // 16) * 10 + 10] = 1.0
    selb = np.zeros((B_LOC, NBB, 128), dtype=bf)
    for bb in range(NBB):
        for p in range(128):
            selb[bb * 8 + p // 16, bb, p] = 1.0

    in_maps = []
    for c in range(N_CORES):
        xc = np.asarray(x[c * B_LOC:(c + 1) * B_LOC], dtype=np.float32)
        # bd[e, q=(ii,k), j, bb, m=(bp,ii')] = x[bb*8+bp, (e*9+j)*16+ii, k] iff ii'==ii
        r = xc.reshape(NBB, 8, 8, 9, 16, ID)          # [bb, bp, e, j, ii, k]
        bd6 = np.zeros((8, 16, ID, 9, NBB, 8, 16), dtype=np.float32)
        for ii in range(16):
            bd6[:, ii, :, :, :, :, ii] = r[:, :, :, :, ii, :].transpose(2, 4, 3, 0, 1)
        bd = np.ascontiguousarray(bd6.reshape(8, 128, 9, NBB, 128)).astype(f8)
        # xt[q=(ii,k), iblk, b] = x[b, iblk*16+ii, k]
        xt = np.ascontiguousarray(
            xc.reshape(B_LOC, NIB, 16, ID).transpose(2, 3, 1, 0)
            .reshape(128, NIB, B_LOC)).astype(bf)
        in_maps.append(
            {"bd": bd, "xt": xt, "ws": ws, "msk": msk, "o80": o80, "sel": sel,
             "mk80": mk80, "selb": selb})
    return in_maps


_NC_CACHE = {}


def kernel(x, W):
    from concourse.bass_utils import run_bass_kernel_spmd

    if "nc" not in _NC_CACHE:
        _NC_CACHE["nc"] = build_program()
    nc = _NC_CACHE["nc"]
    in_maps = _host_inputs(x, W)
    res = run_bass_kernel_spmd(nc, in_maps, core_ids=list(range(N_CORES)))
    out = np.concatenate([r["out"] for r in res.results], axis=0)
    return out.reshape(B_FULL, OC, OD).astype(np.float32)


if __name__ == "__main__":
    nc = build_program()
    print("program built ok")


# revision 8
# speedup vs baseline: 1.0535x; 1.0535x over previous
"""Trainium2 Bass kernel for a CapsuleNet dynamic-routing layer (v2).

Math (per batch element b):
    u_hat[b,i,o,d] = sum_k W[i,o,d,k] * x[b,i,k]      # B=256, IC=1152, OC=10, OD=16, ID=8
    b_log = 0
    for it in 0..2:
        c = softmax(b_log, axis=o)
        s[b,o,d] = sum_i c[b,i,o] * u_hat[b,i,o,d]
        v = squash(s)
        if it < 2: b_log += sum_d u_hat * v

Sharding: data-parallel over B across 8 cores (32 local rows), W replicated.

v2 changes vs v1 (cost-model driven):
  - W loaded once (one HWDGE DMA) instead of 16 Pool-queue DMAs.
  - iter-0 s = 0.1*sum_i u_hat computed as ONE accumulating GEMM chain
    (72 matmuls, M=32) instead of 288 M=8 matmuls.
  - bd (block-diag x) shipped as fp8e4: halves the dominant HBM load.
  - c block-diag scatter moved from Pool-queue (994ns prep each) to
    SP HWDGE queues.
  - PSUM drains split ACT/DVE/Pool; agreement split DVE ~79% / Pool ~21%
    (DVE 2x mode = 0.52 ns/col vs Pool 1.98).
  - blg stored o-last so agreement writes are packed (2x); softmax exp
    transposes to j-last via the ACT engine (free there) so the scatter
    DMA sees 144B contiguous runs.
"""

import sys

sys.path.insert(0, "/opt/trn_rl_repo")

from contextlib import ExitStack

import numpy as np

import concourse.bass as bass
import concourse.tile as tile
from concourse import mybir

BF = mybir.dt.float16
F8 = mybir.dt.float8e4
F32 = mybir.dt.float32
import os
BD_DT = F8 if os.environ.get("BD_FP8", "0") == "1" else BF
AX = mybir.AxisListType
AF = mybir.ActivationFunctionType

N_CORES = 8
B_FULL, IC, OC, OD, ID = 256, 1152, 10, 16, 8
B_LOC = B_FULL // N_CORES          # 32
NIB = IC // 16                     # 72 i-blocks of 16
NBB = B_LOC // 8                   # 4 b-blocks of 8
F = OC * OD                        # 160

# agreement j-slices: DVE gets 3 slices, Pool one (rate-balanced 79/21)
AGR_SLICES = [(0, 19), (19, 38), (38, 57), (57, 72)]


def _squash(nc, smp, ps, scale, vout):
    """vout = squash(scale * ps) with ps an [P, 160] psum slab (f32).

    squash(s) = (n^2/(1+n^2)) * s/(n + 1e-8),  n = ||s||_2 over d.
    The 1e-8 is dropped (n >= 1e-3 in routing; rel err < 1e-5).
    """
    P = ps.shape[0]
    sq = smp.tile([P, F], F32, tag="sq")
    nc.scalar.activation(sq[:], ps[:], AF.Square, scale=float(scale))
    n2 = smp.tile([P, OC], F32, tag="n2")
    nc.vector.tensor_reduce(
        n2[:], sq[:].rearrange("p (o d) -> p o d", d=OD), axis=AX.X,
        op=mybir.AluOpType.add)
    n1 = smp.tile([P, OC], F32, tag="n1")
    nc.vector.tensor_scalar_add(n1[:], n2[:], 1.0)
    sn = smp.tile([P, OC], F32, tag="sn")
    nc.scalar.sqrt(sn[:], n2[:])
    t1 = smp.tile([P, OC], F32, tag="t1")
    nc.vector.tensor_mul(t1[:], n1[:], sn[:])
    r1 = smp.tile([P, OC], F32, tag="r1")
    nc.vector.reciprocal(r1[:], t1[:])
    f1 = smp.tile([P, OC], F32, tag="f1")
    nc.vector.tensor_mul(f1[:], n2[:], r1[:])
    if scale != 1.0:
        nc.vector.tensor_scalar_mul(f1[:], f1[:], float(scale))
    nc.vector.tensor_mul(
        vout[:].rearrange("p (o d) -> p o d", d=OD),
        ps[:].rearrange("p (o d) -> p o d", d=OD),
        f1[:].unsqueeze(-1).broadcast_to((P, OC, OD)))


def _split_multiwait(nc):
    """Walrus encodes at most ONE semaphore wait per engine/DMA instruction.
    Hoist excess waits onto same-engine NoOps placed directly before the
    instruction.  HWDGE DMAs can't be gated that way - assert instead."""
    for fn in nc.m.functions:
        for bb in fn.blocks:
            out = []
            k = 0
            for ins in bb.instructions:
                si = ins.sync_info
                waits = list(si.on_wait) if si is not None and si.on_wait else []
                limit = 1
                if ins.opcode == "DMACopy":
                    q = str(getattr(ins, "queue", "") or "")
                    if "HW" in q and len(waits) > 1:
                        raise AssertionError(
                            f"HWDGE DMA {ins.name} has {len(waits)} waits: {ins}")
                if len(waits) > limit:
                    for w in waits[:-limit]:
                        nop = mybir.InstNoOp(name=f"{ins.name}-wn{k}", ins=[], outs=[])
                        k += 1
                        nop.engine = ins.engine
                        nop.sync_info = mybir.SyncInfo(on_wait=[w], on_update=[])
                        out.append(nop)
                    ins.sync_info = mybir.SyncInfo(
                        on_wait=waits[-limit:],
                        on_update=list(si.on_update) if si.on_update else [])
                out.append(ins)
            bb.instructions = out


def build_program(split_waits=True):
    nc = bass.Bass()
    bd_d = nc.declare_dram_parameter("bd", [8, 128, 9, NBB, 128], BD_DT, isOutput=False)
    xt_d = nc.declare_dram_parameter("xt", [128, NIB, B_LOC], BF, isOutput=False)
    ws_d = nc.declare_dram_parameter("ws", [128, NIB, F], BF, isOutput=False)
    msk_d = nc.declare_dram_parameter("msk", [80, F], BF, isOutput=False)
    o80_d = nc.declare_dram_parameter("o80", [80, 8], BF, isOutput=False)
    sel_d = nc.declare_dram_parameter("sel", [8, 128], BF, isOutput=False)
    selb_d = nc.declare_dram_parameter("selb", [B_LOC, NBB, 128], BF, isOutput=False)
    mk80_d = nc.declare_dram_parameter("mk80", [128, 80], BF, isOutput=False)
    out_d = nc.declare_dram_parameter("out", [B_LOC, F], F32, isOutput=True)

    with ExitStack() as ctx:
        tc = ctx.enter_context(tile.TileContext(nc))
        st = ctx.enter_context(tc.tile_pool(name="st", bufs=1))
        bdp = ctx.enter_context(tc.tile_pool(name="bdp", bufs=2))
        y2p = ctx.enter_context(tc.tile_pool(name="y2p", bufs=1))
        tsp = ctx.enter_context(tc.tile_pool(name="tsp", bufs=1))
        mkp = ctx.enter_context(tc.tile_pool(name="mkp", bufs=2))
        cnp = ctx.enter_context(tc.tile_pool(name="cnp", bufs=1))
        cbp = ctx.enter_context(tc.tile_pool(name="cbp", bufs=1))
        vxp = ctx.enter_context(tc.tile_pool(name="vxp", bufs=2))
        smp = ctx.enter_context(tc.tile_pool(name="smp", bufs=4))
        pbig = ctx.enter_context(tc.tile_pool(name="pbig", bufs=3, space="PSUM"))
        ps1p = ctx.enter_context(tc.tile_pool(name="ps1p", bufs=2, space="PSUM"))
        ps0p = ctx.enter_context(tc.tile_pool(name="ps0p", bufs=1, space="PSUM"))
        pvxp = ctx.enter_context(tc.tile_pool(name="pvxp", bufs=1, space="PSUM"))
        psvp = ctx.enter_context(tc.tile_pool(name="psvp", bufs=1, space="PSUM"))

        # --- persistent tiles ---
        u_hat = st.tile([128, NIB, NBB, F], BF, tag="u_hat")
        ws_sb = st.tile([128, NIB, F], BF, tag="ws_sb")
        xt_sb = st.tile([128, NIB, B_LOC], BF, tag="xt_sb")
        blg = st.tile([128, NBB, NIB, OC], BF, tag="blg")
        c_sb = st.tile([128, NBB, NIB, OC], BF, tag="c_sb")
        msk_sb = st.tile([80, F], BF, tag="msk_sb")
        o80_sb = st.tile([80, 8], BF, tag="o80_sb")
        sel_sb = st.tile([8, 128], BF, tag="sel_sb")
        selb_sb = st.tile([B_LOC, NBB, 128], BF, tag="selb_sb")
        mk80_sb = st.tile([128, 80], BF, tag="mk80_sb")
        v8 = [st.tile([8, F], BF, tag=f"v8_{i}", name=f"v8_{i}") for i in range(NBB)]
        vx4 = [st.tile([128, F], BF, tag=f"vx{i}", name=f"vx{i}") for i in range(NBB)]
        vall = st.tile([B_LOC, F], BF, tag="vall")
        og = st.tile([8, NBB, F], F32, tag="og")

        # --- input loads + zero-init of the c-blockdiag ---
        nc.sync.dma_start(out=xt_sb[:], in_=xt_d[:])
        for wc in range(4):
            nc.scalar.dma_start(
                out=ws_sb[:, wc * 18:(wc + 1) * 18, :],
                in_=ws_d[:, wc * 18:(wc + 1) * 18, :])
        nc.sync.dma_start(out=selb_sb[:], in_=selb_d[:])
        nc.sync.dma_start(out=sel_sb[:], in_=sel_d[:])
        nc.scalar.dma_start(out=msk_sb[:], in_=msk_d[:])
        nc.scalar.dma_start(out=o80_sb[:], in_=o80_d[:])
        nc.sync.dma_start(out=mk80_sb[:], in_=mk80_d[:])

        # --- iter-0 stage 1: s0 = 0.1*sum_i u_hat as one GEMM chain over the
        # full (i,k) contraction: xt[128,(iblk),32] x ws[128,(iblk),160] ---
        ps0 = ps0p.tile([B_LOC, F], F32, tag="ps0", name="ps0")
        for j in range(NIB):
            nc.tensor.matmul(
                ps0[:], lhsT=xt_sb[:, j, :], rhs=ws_sb[:, j, :],
                start=(j == 0), stop=(j == NIB - 1))

        def load_vx(bblk, first):
            pvx = pvxp.tile([128, F], F32, tag="pvx")
            if first:
                nc.tensor.matmul(
                    pvx[:], lhsT=selb_sb[:, bblk, :], rhs=vall[:],
                    start=True, stop=True)
            else:
                nc.tensor.matmul(
                    pvx[:], lhsT=sel_sb[:], rhs=v8[bblk][:],
                    start=True, stop=True)
            nc.scalar.copy(vx4[bblk][:], pvx[:])

        def agreement_slice(bblk, j0, j1, first, eng):
            """blg[:, bblk, j0:j1, :] (+)= sum_d u_hat * v for a j-slice."""
            w = j1 - j0
            pool = eng is nc.gpsimd
            y2 = y2p.tile([128, w, F], BF, tag="y2p3" if pool else "y2")
            eng.tensor_mul(
                y2[:], u_hat[:, j0:j1, bblk, :],
                vx4[bblk][:].unsqueeze(1).broadcast_to((128, w, F)))
            # in-place pairwise-add tree over d (fp16, 2x mode)
            y2v = y2[:].rearrange("p j (o d) -> p j o d", d=OD)
            eng.tensor_add(
                y2v[:, :, :, 0:8], y2v[:, :, :, 0:8], y2v[:, :, :, 8:16])
            eng.tensor_add(
                y2v[:, :, :, 0:4], y2v[:, :, :, 0:4], y2v[:, :, :, 4:8])
            eng.tensor_add(
                y2v[:, :, :, 0:2], y2v[:, :, :, 0:2], y2v[:, :, :, 2:4])
            dst = blg[:, bblk, j0:j1, :]
            if first:
                eng.tensor_add(dst, y2v[:, :, :, 0], y2v[:, :, :, 1])
            else:
                ts = tsp.tile([128, w, OC], BF, tag="tsp3" if pool else "ts")
                eng.tensor_add(ts[:], y2v[:, :, :, 0], y2v[:, :, :, 1])
                eng.tensor_add(dst, dst, ts[:])


        # --- u_hat build: one matmul per (iblk, bblk), K=128=(ii,k), N=160;
        # PSUM drained to SBUF across ACT/DVE/Pool.  Iter-0 squash runs first
        # so each e-chunk's agreement slices interleave with the build. ---
        _squash(nc, smp, ps0[:], 0.1, vall)
        for bblk in range(NBB):
            load_vx(bblk, first=True)
        drain_seq = [nc.scalar, nc.scalar, nc.scalar, nc.vector, nc.scalar,
                     nc.scalar, nc.vector, nc.scalar, nc.scalar, nc.vector]
        ndrain = 0
        nagr = 0
        for e in range(8):
            bdt = bdp.tile([128, 9, NBB, 128], BD_DT, tag="bdt")
            nc.gpsimd.dma_start(out=bdt[:], in_=bd_d[e])
            for j in range(9):
                iblk = e * 9 + j
                for h in range(2):
                    ps = pbig.tile([128, 2, F], F32, tag="pbig")
                    for bb in range(2):
                        nc.tensor.matmul(
                            ps[:, bb, :], lhsT=bdt[:, j, h * 2 + bb, :],
                            rhs=ws_sb[:, iblk, :], start=True, stop=True)
                    eng = drain_seq[ndrain % len(drain_seq)]
                    ndrain += 1
                    if eng is nc.scalar:
                        eng.copy(u_hat[:, iblk, h * 2:(h + 1) * 2, :], ps[:])
                    else:
                        eng.tensor_copy(u_hat[:, iblk, h * 2:(h + 1) * 2, :], ps[:])
            # iter-0 agreement for this e's 9 j-columns, all b-blocks
            for bblk in range(NBB):
                eng = nc.gpsimd if nagr % 4 == 3 else nc.vector
                nagr += 1
                agreement_slice(bblk, e * 9, e * 9 + 9, True, eng)

        # --- iters 1, 2: software-pipelined across bblks and iterations.
        # softmax+cbt feed the PE stage-1 stream; each bblk's iter-2 chain
        # starts as soon as its iter-1 agreement lands. ---
        def softmax_cbt(it, bblk):
            nc.scalar.activation(
                c_sb[:, bblk, :, :], blg[:, bblk, :, :], AF.Exp)
            sm = smp.tile([128, NIB], F32, tag="sm")
            nc.vector.tensor_reduce(
                sm[:], c_sb[:, bblk, :, :], axis=AX.X, op=mybir.AluOpType.add)
            rr = smp.tile([128, NIB], BF, tag="rr")
            with nc.allow_low_precision(reason="softmax denom recip in fp16"):
                nc.vector.reciprocal(rr[:], sm[:])
            cn = cnp.tile([128, NIB, OC], BF, tag="cn")
            nc.vector.tensor_mul(
                cn[:], c_sb[:, bblk, :, :],
                rr[:].unsqueeze(-1).broadcast_to((128, NIB, OC)))
            eng = nc.vector
            cbt = cbp.tile([128, NIB, 80], BF, tag=f"cbt{bblk % 2}")
            eng.tensor_mul(
                cbt[:],
                cn[:].unsqueeze(2).broadcast_to((128, NIB, 8, OC)),
                mk80_sb[:].rearrange("p (b o) -> p b o", o=OC).unsqueeze(1)
                .broadcast_to((128, NIB, 8, OC)))
            ps1 = ps1p.tile([80, F], F32, tag="ps1", name=f"ps1_{it}_{bblk}")
            for j in range(NIB):
                nc.tensor.matmul(
                    ps1[:], lhsT=cbt[:, j, :],
                    rhs=u_hat[:, j, bblk, :],
                    start=(j == 0), stop=(j == NIB - 1))
            return ps1

        def extract_s(ps1):
            mskd = mkp.tile([80, F], BF, tag="mskd")
            nc.vector.tensor_mul(mskd[:], ps1[:], msk_sb[:])
            psv = psvp.tile([8, F], F32, tag="psv")
            nc.tensor.matmul(
                psv[:], lhsT=o80_sb[:], rhs=mskd[:], start=True, stop=True)
            return psv

        for it in (1, 2):
            ps1s = [softmax_cbt(it, bb) for bb in range(NBB)]
            for bblk in range(NBB):
                psv = extract_s(ps1s[bblk])
                if it == 1:
                    _squash(nc, smp, psv[:], 1.0, v8[bblk])
                    load_vx(bblk, first=False)
                    agreement_slice(bblk, 0, 25, False, nc.vector)
                    agreement_slice(bblk, 25, 50, False, nc.vector)
                    agreement_slice(bblk, 50, 72, False, nc.gpsimd)
                else:
                    _squash(nc, smp, psv[:], 1.0, og[:, bblk, :])
        nc.gpsimd.dma_start(
            out=out_d[:].rearrange("(bb bp) f -> bp bb f", bp=8), in_=og[:])

    if split_waits:
        _split_multiwait(nc)
    return nc


def _host_inputs(x, W):
    """Per-core input maps from full x [256,1152,8] f32, W [1,1152,10,16,8] f32."""
    bf = np.float16
    f8 = mybir.dt.np(BD_DT)
    W0 = np.asarray(W[0], dtype=np.float32)
    # ws[q=(ii,k), j, (o,d)] = W[j*16+ii, o, d, k]
    ws = np.ascontiguousarray(
        W0.reshape(NIB, 16, OC, OD, ID).transpose(1, 4, 0, 2, 3)
        .reshape(128, NIB, F)).astype(bf)
    msk = np.zeros((80, F), dtype=bf)
    for bpp in range(8):
        for o in range(OC):
            msk[bpp * 10 + o, o * OD:(o + 1) * OD] = 1.0
    o80 = np.zeros((80, 8), dtype=bf)
    for p in range(80):
        o80[p, p // 10] = 1.0
    sel = np.zeros((8, 128), dtype=bf)
    for p in range(128):
        sel[p // 16, p] = 1.0
    mk80 = np.zeros((128, 80), dtype=bf)
    for p in range(128):
        mk80[p, (p // 16) * 10:(p // 16) * 10 + 10] = 1.0
    selb = np.zeros((B_LOC, NBB, 128), dtype=bf)
    for bb in range(NBB):
        for p in range(128):
            selb[bb * 8 + p // 16, bb, p] = 1.0

    in_maps = []
    for c in range(N_CORES):
        xc = np.asarray(x[c * B_LOC:(c + 1) * B_LOC], dtype=np.float32)
        # bd[e, q=(ii,k), j, bb, m=(bp,ii')] = x[bb*8+bp, (e*9+j)*16+ii, k] iff ii'==ii
        r = xc.reshape(NBB, 8, 8, 9, 16, ID)          # [bb, bp, e, j, ii, k]
        bd6 = np.zeros((8, 16, ID, 9, NBB, 8, 16), dtype=np.float32)
        for ii in range(16):
            bd6[:, ii, :, :, :, :, ii] = r[:, :, :, :, ii, :].transpose(2, 4, 3, 0, 1)
        bd = np.ascontiguousarray(bd6.reshape(8, 128, 9, NBB, 128)).astype(f8)
        # xt[q=(ii,k), iblk, b] = x[b, iblk*16+ii, k]
        xt = np.ascontiguousarray(
            xc.reshape(B_LOC, NIB, 16, ID).transpose(2, 3, 1, 0)
            .reshape(128, NIB, B_LOC)).astype(bf)
        in_maps.append(
            {"bd": bd, "xt": xt, "ws": ws, "msk": msk, "o80": o80, "sel": sel,
             "mk80": mk80, "selb": selb})
    return in_maps


_NC_CACHE = {}


def kernel(x, W):
    from concourse.bass_utils import run_bass_kernel_spmd

    if "nc" not in _NC_CACHE:
        _NC_CACHE["nc"] = build_program()
    nc = _NC_CACHE["nc"]
    in_maps = _host_inputs(x, W)
    res = run_bass_kernel_spmd(nc, in_maps, core_ids=list(range(N_CORES)))
    out = np.concatenate([r["out"] for r in res.results], axis=0)
    return out.reshape(B_FULL, OC, OD).astype(np.float32)


if __name__ == "__main__":
    nc = build_program()
    print("program built ok")


# revision 9
# speedup vs baseline: 1.0900x; 1.0347x over previous
"""Trainium2 Bass kernel for a CapsuleNet dynamic-routing layer (v2).

Math (per batch element b):
    u_hat[b,i,o,d] = sum_k W[i,o,d,k] * x[b,i,k]      # B=256, IC=1152, OC=10, OD=16, ID=8
    b_log = 0
    for it in 0..2:
        c = softmax(b_log, axis=o)
        s[b,o,d] = sum_i c[b,i,o] * u_hat[b,i,o,d]
        v = squash(s)
        if it < 2: b_log += sum_d u_hat * v

Sharding: data-parallel over B across 8 cores (32 local rows), W replicated.

v2 changes vs v1 (cost-model driven):
  - W loaded once (one HWDGE DMA) instead of 16 Pool-queue DMAs.
  - iter-0 s = 0.1*sum_i u_hat computed as ONE accumulating GEMM chain
    (72 matmuls, M=32) instead of 288 M=8 matmuls.
  - bd (block-diag x) shipped as fp8e4: halves the dominant HBM load.
  - c block-diag scatter moved from Pool-queue (994ns prep each) to
    SP HWDGE queues.
  - PSUM drains split ACT/DVE/Pool; agreement split DVE ~79% / Pool ~21%
    (DVE 2x mode = 0.52 ns/col vs Pool 1.98).
  - blg stored o-last so agreement writes are packed (2x); softmax exp
    transposes to j-last via the ACT engine (free there) so the scatter
    DMA sees 144B contiguous runs.
"""

import sys

sys.path.insert(0, "/opt/trn_rl_repo")

from contextlib import ExitStack

import numpy as np

import concourse.bass as bass
import concourse.tile as tile
from concourse import mybir

BF = mybir.dt.float16
F8 = mybir.dt.float8e4
F32 = mybir.dt.float32
import os
BD_DT = F8 if os.environ.get("BD_FP8", "0") == "1" else BF
AX = mybir.AxisListType
AF = mybir.ActivationFunctionType

N_CORES = 8
B_FULL, IC, OC, OD, ID = 256, 1152, 10, 16, 8
B_LOC = B_FULL // N_CORES          # 32
NIB = IC // 16                     # 72 i-blocks of 16
NBB = B_LOC // 8                   # 4 b-blocks of 8
F = OC * OD                        # 160

# agreement j-slices: DVE gets 3 slices, Pool one (rate-balanced 79/21)
AGR_SLICES = [(0, 19), (19, 38), (38, 57), (57, 72)]


def _squash(nc, smp, ps, scale, vout):
    """vout = squash(scale * ps) with ps an [P, 160] psum slab (f32).

    squash(s) = (n^2/(1+n^2)) * s/(n + 1e-8),  n = ||s||_2 over d.
    The 1e-8 is dropped (n >= 1e-3 in routing; rel err < 1e-5).
    """
    P = ps.shape[0]
    sq = smp.tile([P, F], F32, tag="sq")
    nc.scalar.activation(sq[:], ps[:], AF.Square, scale=float(scale))
    n2 = smp.tile([P, OC], F32, tag="n2")
    nc.vector.tensor_reduce(
        n2[:], sq[:].rearrange("p (o d) -> p o d", d=OD), axis=AX.X,
        op=mybir.AluOpType.add)
    n1 = smp.tile([P, OC], F32, tag="n1")
    nc.vector.tensor_scalar_add(n1[:], n2[:], 1.0)
    sn = smp.tile([P, OC], F32, tag="sn")
    nc.scalar.sqrt(sn[:], n2[:])
    t1 = smp.tile([P, OC], F32, tag="t1")
    nc.vector.tensor_mul(t1[:], n1[:], sn[:])
    r1 = smp.tile([P, OC], F32, tag="r1")
    nc.vector.reciprocal(r1[:], t1[:])
    f1 = smp.tile([P, OC], F32, tag="f1")
    nc.vector.tensor_mul(f1[:], n2[:], r1[:])
    if scale != 1.0:
        nc.vector.tensor_scalar_mul(f1[:], f1[:], float(scale))
    nc.vector.tensor_mul(
        vout[:].rearrange("p (o d) -> p o d", d=OD),
        ps[:].rearrange("p (o d) -> p o d", d=OD),
        f1[:].unsqueeze(-1).broadcast_to((P, OC, OD)))


def _split_multiwait(nc):
    """Walrus encodes at most ONE semaphore wait per engine/DMA instruction.
    Hoist excess waits onto same-engine NoOps placed directly before the
    instruction.  HWDGE DMAs can't be gated that way - assert instead."""
    for fn in nc.m.functions:
        for bb in fn.blocks:
            out = []
            k = 0
            for ins in bb.instructions:
                si = ins.sync_info
                waits = list(si.on_wait) if si is not None and si.on_wait else []
                limit = 1
                if ins.opcode == "DMACopy":
                    q = str(getattr(ins, "queue", "") or "")
                    if "HW" in q and len(waits) > 1:
                        raise AssertionError(
                            f"HWDGE DMA {ins.name} has {len(waits)} waits: {ins}")
                if len(waits) > limit:
                    for w in waits[:-limit]:
                        nop = mybir.InstNoOp(name=f"{ins.name}-wn{k}", ins=[], outs=[])
                        k += 1
                        nop.engine = ins.engine
                        nop.sync_info = mybir.SyncInfo(on_wait=[w], on_update=[])
                        out.append(nop)
                    ins.sync_info = mybir.SyncInfo(
                        on_wait=waits[-limit:],
                        on_update=list(si.on_update) if si.on_update else [])
                out.append(ins)
            bb.instructions = out


def build_program(split_waits=True):
    nc = bass.Bass()
    bd_d = nc.declare_dram_parameter("bd", [8, 128, 9, NBB, 128], BD_DT, isOutput=False)
    xt_d = nc.declare_dram_parameter("xt", [128, NIB, B_LOC], BF, isOutput=False)
    ws_d = nc.declare_dram_parameter("ws", [128, NIB, F], BF, isOutput=False)
    msk_d = nc.declare_dram_parameter("msk", [80, F], BF, isOutput=False)
    o80_d = nc.declare_dram_parameter("o80", [80, 8], BF, isOutput=False)
    sel_d = nc.declare_dram_parameter("sel", [8, 128], BF, isOutput=False)
    selb_d = nc.declare_dram_parameter("selb", [B_LOC, NBB, 128], BF, isOutput=False)
    mk80_d = nc.declare_dram_parameter("mk80", [128, 80], BF, isOutput=False)
    out_d = nc.declare_dram_parameter("out", [B_LOC, F], F32, isOutput=True)

    with ExitStack() as ctx:
        tc = ctx.enter_context(tile.TileContext(nc))
        st = ctx.enter_context(tc.tile_pool(name="st", bufs=1))
        bdp = ctx.enter_context(tc.tile_pool(name="bdp", bufs=2))
        y2p = ctx.enter_context(tc.tile_pool(name="y2p", bufs=1))
        tsp = ctx.enter_context(tc.tile_pool(name="tsp", bufs=1))
        mkp = ctx.enter_context(tc.tile_pool(name="mkp", bufs=2))
        cnp = ctx.enter_context(tc.tile_pool(name="cnp", bufs=1))
        cbp = ctx.enter_context(tc.tile_pool(name="cbp", bufs=1))
        vxp = ctx.enter_context(tc.tile_pool(name="vxp", bufs=2))
        smp = ctx.enter_context(tc.tile_pool(name="smp", bufs=4))
        pbig = ctx.enter_context(tc.tile_pool(name="pbig", bufs=3, space="PSUM"))
        ps1p = ctx.enter_context(tc.tile_pool(name="ps1p", bufs=2, space="PSUM"))
        ps0p = ctx.enter_context(tc.tile_pool(name="ps0p", bufs=1, space="PSUM"))
        pvxp = ctx.enter_context(tc.tile_pool(name="pvxp", bufs=1, space="PSUM"))
        psvp = ctx.enter_context(tc.tile_pool(name="psvp", bufs=1, space="PSUM"))

        # --- persistent tiles ---
        u_hat = st.tile([128, NIB, NBB, F], BF, tag="u_hat")
        ws_sb = st.tile([128, NIB, F], BF, tag="ws_sb")
        xt_sb = st.tile([128, NIB, B_LOC], BF, tag="xt_sb")
        blg = st.tile([128, NBB, NIB, OC], BF, tag="blg")
        c_sb = st.tile([128, NBB, NIB, OC], BF, tag="c_sb")
        msk_sb = st.tile([80, F], BF, tag="msk_sb")
        o80_sb = st.tile([80, 8], BF, tag="o80_sb")
        sel_sb = st.tile([8, 128], BF, tag="sel_sb")
        selb_sb = st.tile([B_LOC, NBB, 128], BF, tag="selb_sb")
        mk80_sb = st.tile([128, 80], BF, tag="mk80_sb")
        v8 = [st.tile([8, F], BF, tag=f"v8_{i}", name=f"v8_{i}") for i in range(NBB)]
        vx4 = [st.tile([128, F], BF, tag=f"vx{i}", name=f"vx{i}") for i in range(NBB)]
        vall = st.tile([B_LOC, F], BF, tag="vall")
        og = st.tile([8, NBB, F], F32, tag="og")

        # --- input loads + zero-init of the c-blockdiag ---
        nc.sync.dma_start(out=xt_sb[:], in_=xt_d[:])
        for wc in range(4):
            nc.scalar.dma_start(
                out=ws_sb[:, wc * 18:(wc + 1) * 18, :],
                in_=ws_d[:, wc * 18:(wc + 1) * 18, :])
        nc.sync.dma_start(out=selb_sb[:], in_=selb_d[:])
        nc.sync.dma_start(out=sel_sb[:], in_=sel_d[:])
        nc.scalar.dma_start(out=msk_sb[:], in_=msk_d[:])
        nc.scalar.dma_start(out=o80_sb[:], in_=o80_d[:])
        nc.sync.dma_start(out=mk80_sb[:], in_=mk80_d[:])

        # --- iter-0 stage 1: s0 = 0.1*sum_i u_hat as one GEMM chain over the
        # full (i,k) contraction: xt[128,(iblk),32] x ws[128,(iblk),160] ---
        ps0 = ps0p.tile([B_LOC, F], F32, tag="ps0", name="ps0")
        for j in range(NIB):
            nc.tensor.matmul(
                ps0[:], lhsT=xt_sb[:, j, :], rhs=ws_sb[:, j, :],
                start=(j == 0), stop=(j == NIB - 1))

        def load_vx(bblk, first):
            pvx = pvxp.tile([128, F], F32, tag="pvx")
            if first:
                nc.tensor.matmul(
                    pvx[:], lhsT=selb_sb[:, bblk, :], rhs=vall[:],
                    start=True, stop=True)
            else:
                nc.tensor.matmul(
                    pvx[:], lhsT=sel_sb[:], rhs=v8[bblk][:],
                    start=True, stop=True)
            nc.scalar.copy(vx4[bblk][:], pvx[:])

        def agreement_slice(bblk, j0, j1, first, eng):
            """blg[:, bblk, j0:j1, :] (+)= sum_d u_hat * v for a j-slice."""
            w = j1 - j0
            pool = eng is nc.gpsimd
            y2 = y2p.tile([128, w, F], BF, tag="y2p3" if pool else "y2")
            eng.tensor_mul(
                y2[:], u_hat[:, j0:j1, bblk, :],
                vx4[bblk][:].unsqueeze(1).broadcast_to((128, w, F)))
            # in-place pairwise-add tree over d (fp16, 2x mode)
            y2v = y2[:].rearrange("p j (o d) -> p j o d", d=OD)
            eng.tensor_add(
                y2v[:, :, :, 0:8], y2v[:, :, :, 0:8], y2v[:, :, :, 8:16])
            eng.tensor_add(
                y2v[:, :, :, 0:4], y2v[:, :, :, 0:4], y2v[:, :, :, 4:8])
            eng.tensor_add(
                y2v[:, :, :, 0:2], y2v[:, :, :, 0:2], y2v[:, :, :, 2:4])
            dst = blg[:, bblk, j0:j1, :]
            if first:
                eng.tensor_add(dst, y2v[:, :, :, 0], y2v[:, :, :, 1])
            else:
                ts = tsp.tile([128, w, OC], BF, tag="tsp3" if pool else "ts")
                eng.tensor_add(ts[:], y2v[:, :, :, 0], y2v[:, :, :, 1])
                eng.tensor_add(dst, dst, ts[:])


        # --- u_hat build: one matmul per (iblk, bblk), K=128=(ii,k), N=160;
        # PSUM drained to SBUF across ACT/DVE/Pool.  Iter-0 squash runs first
        # so each e-chunk's agreement slices interleave with the build. ---
        _squash(nc, smp, ps0[:], 0.1, vall)
        for bblk in range(NBB):
            load_vx(bblk, first=True)
        drain_seq = [nc.scalar, nc.scalar, nc.scalar, nc.scalar, nc.scalar,
                     nc.scalar, nc.scalar, nc.scalar, nc.scalar, nc.vector]
        ndrain = 0
        nagr = 0
        for e in range(8):
            bdt = bdp.tile([128, 9, NBB, 128], BD_DT, tag="bdt")
            nc.gpsimd.dma_start(out=bdt[:], in_=bd_d[e])
            for j in range(9):
                iblk = e * 9 + j
                for h in range(2):
                    ps = pbig.tile([128, 2, F], F32, tag="pbig")
                    for bb in range(2):
                        nc.tensor.matmul(
                            ps[:, bb, :], lhsT=bdt[:, j, h * 2 + bb, :],
                            rhs=ws_sb[:, iblk, :], start=True, stop=True)
                    eng = drain_seq[ndrain % len(drain_seq)]
                    ndrain += 1
                    if eng is nc.scalar:
                        eng.copy(u_hat[:, iblk, h * 2:(h + 1) * 2, :], ps[:])
                    else:
                        eng.tensor_copy(u_hat[:, iblk, h * 2:(h + 1) * 2, :], ps[:])
            # iter-0 agreement for this e's 9 j-columns, all b-blocks
            for bblk in range(NBB):
                eng = nc.gpsimd if nagr % 4 == 3 else nc.vector
                nagr += 1
                agreement_slice(bblk, e * 9, e * 9 + 9, True, eng)

        # --- iters 1, 2: software-pipelined across bblks and iterations.
        # softmax+cbt feed the PE stage-1 stream; each bblk's iter-2 chain
        # starts as soon as its iter-1 agreement lands. ---
        def softmax_cbt(it, bblk):
            nc.scalar.activation(
                c_sb[:, bblk, :, :], blg[:, bblk, :, :], AF.Exp)
            sm = smp.tile([128, NIB], F32, tag="sm")
            nc.vector.tensor_reduce(
                sm[:], c_sb[:, bblk, :, :], axis=AX.X, op=mybir.AluOpType.add)
            rr = smp.tile([128, NIB], BF, tag="rr")
            with nc.allow_low_precision(reason="softmax denom recip in fp16"):
                nc.vector.reciprocal(rr[:], sm[:])
            cn = cnp.tile([128, NIB, OC], BF, tag="cn")
            nc.vector.tensor_mul(
                cn[:], c_sb[:, bblk, :, :],
                rr[:].unsqueeze(-1).broadcast_to((128, NIB, OC)))
            eng = nc.vector
            cbt = cbp.tile([128, NIB, 80], BF, tag=f"cbt{bblk % 2}")
            eng.tensor_mul(
                cbt[:],
                cn[:].unsqueeze(2).broadcast_to((128, NIB, 8, OC)),
                mk80_sb[:].rearrange("p (b o) -> p b o", o=OC).unsqueeze(1)
                .broadcast_to((128, NIB, 8, OC)))
            ps1 = ps1p.tile([80, F], F32, tag="ps1", name=f"ps1_{it}_{bblk}")
            for j in range(NIB):
                nc.tensor.matmul(
                    ps1[:], lhsT=cbt[:, j, :],
                    rhs=u_hat[:, j, bblk, :],
                    start=(j == 0), stop=(j == NIB - 1))
            return ps1

        def extract_s(ps1):
            mskd = mkp.tile([80, F], BF, tag="mskd")
            nc.vector.tensor_mul(mskd[:], ps1[:], msk_sb[:])
            psv = psvp.tile([8, F], F32, tag="psv")
            nc.tensor.matmul(
                psv[:], lhsT=o80_sb[:], rhs=mskd[:], start=True, stop=True)
            return psv

        for it in (1, 2):
            ps1s = [softmax_cbt(it, bb) for bb in range(NBB)]
            for bblk in range(NBB):
                psv = extract_s(ps1s[bblk])
                if it == 1:
                    _squash(nc, smp, psv[:], 1.0, v8[bblk])
                    load_vx(bblk, first=False)
                    agreement_slice(bblk, 0, 25, False, nc.vector)
                    agreement_slice(bblk, 25, 50, False, nc.vector)
                    agreement_slice(bblk, 50, 72, False, nc.gpsimd)
                else:
                    _squash(nc, smp, psv[:], 1.0, og[:, bblk, :])
        nc.gpsimd.dma_start(
            out=out_d[:].rearrange("(bb bp) f -> bp bb f", bp=8), in_=og[:])

    if split_waits:
        _split_multiwait(nc)
    return nc


def _host_inputs(x, W):
    """Per-core input maps from full x [256,1152,8] f32, W [1,1152,10,16,8] f32."""
    bf = np.float16
    f8 = mybir.dt.np(BD_DT)
    W0 = np.asarray(W[0], dtype=np.float32)
    # ws[q=(ii,k), j, (o,d)] = W[j*16+ii, o, d, k]
    ws = np.ascontiguousarray(
        W0.reshape(NIB, 16, OC, OD, ID).transpose(1, 4, 0, 2, 3)
        .reshape(128, NIB, F)).astype(bf)
    msk = np.zeros((80, F), dtype=bf)
    for bpp in range(8):
        for o in range(OC):
            msk[bpp * 10 + o, o * OD:(o + 1) * OD] = 1.0
    o80 = np.zeros((80, 8), dtype=bf)
    for p in range(80):
        o80[p, p // 10] = 1.0
    sel = np.zeros((8, 128), dtype=bf)
    for p in range(128):
        sel[p // 16, p] = 1.0
    mk80 = np.zeros((128, 80), dtype=bf)
    for p in range(128):
        mk80[p, (p // 16) * 10:(p // 16) * 10 + 10] = 1.0
    selb = np.zeros((B_LOC, NBB, 128), dtype=bf)
    for bb in range(NBB):
        for p in range(128):
            selb[bb * 8 + p // 16, bb, p] = 1.0

    in_maps = []
    for c in range(N_CORES):
        xc = np.asarray(x[c * B_LOC:(c + 1) * B_LOC], dtype=np.float32)
        # bd[e, q=(ii,k), j, bb, m=(bp,ii')] = x[bb*8+bp, (e*9+j)*16+ii, k] iff ii'==ii
        r = xc.reshape(NBB, 8, 8, 9, 16, ID)          # [bb, bp, e, j, ii, k]
        bd6 = np.zeros((8, 16, ID, 9, NBB, 8, 16), dtype=np.float32)
        for ii in range(16):
            bd6[:, ii, :, :, :, :, ii] = r[:, :, :, :, ii, :].transpose(2, 4, 3, 0, 1)
        bd = np.ascontiguousarray(bd6.reshape(8, 128, 9, NBB, 128)).astype(f8)
        # xt[q=(ii,k), iblk, b] = x[b, iblk*16+ii, k]
        xt = np.ascontiguousarray(
            xc.reshape(B_LOC, NIB, 16, ID).transpose(2, 3, 1, 0)
            .reshape(128, NIB, B_LOC)).astype(bf)
        in_maps.append(
            {"bd": bd, "xt": xt, "ws": ws, "msk": msk, "o80": o80, "sel": sel,
             "mk80": mk80, "selb": selb})
    return in_maps


_NC_CACHE = {}


def kernel(x, W):
    from concourse.bass_utils import run_bass_kernel_spmd

    if "nc" not in _NC_CACHE:
        _NC_CACHE["nc"] = build_program()
    nc = _NC_CACHE["nc"]
    in_maps = _host_inputs(x, W)
    res = run_bass_kernel_spmd(nc, in_maps, core_ids=list(range(N_CORES)))
    out = np.concatenate([r["out"] for r in res.results], axis=0)
    return out.reshape(B_FULL, OC, OD).astype(np.float32)


if __name__ == "__main__":
    nc = build_program()
    print("program built ok")


# revision 17
# speedup vs baseline: 1.0909x; 1.0008x over previous
"""Trainium2 Bass kernel for a CapsuleNet dynamic-routing layer (v2).

Math (per batch element b):
    u_hat[b,i,o,d] = sum_k W[i,o,d,k] * x[b,i,k]      # B=256, IC=1152, OC=10, OD=16, ID=8
    b_log = 0
    for it in 0..2:
        c = softmax(b_log, axis=o)
        s[b,o,d] = sum_i c[b,i,o] * u_hat[b,i,o,d]
        v = squash(s)
        if it < 2: b_log += sum_d u_hat * v

Sharding: data-parallel over B across 8 cores (32 local rows), W replicated.

v2 changes vs v1 (cost-model driven):
  - W loaded once (one HWDGE DMA) instead of 16 Pool-queue DMAs.
  - iter-0 s = 0.1*sum_i u_hat computed as ONE accumulating GEMM chain
    (72 matmuls, M=32) instead of 288 M=8 matmuls.
  - bd (block-diag x) shipped as fp8e4: halves the dominant HBM load.
  - c block-diag scatter moved from Pool-queue (994ns prep each) to
    SP HWDGE queues.
  - PSUM drains split ACT/DVE/Pool; agreement split DVE ~79% / Pool ~21%
    (DVE 2x mode = 0.52 ns/col vs Pool 1.98).
  - blg stored o-last so agreement writes are packed (2x); softmax exp
    transposes to j-last via the ACT engine (free there) so the scatter
    DMA sees 144B contiguous runs.
"""

import sys

sys.path.insert(0, "/opt/trn_rl_repo")

from contextlib import ExitStack

import numpy as np

import concourse.bass as bass
import concourse.tile as tile
from concourse import mybir

BF = mybir.dt.float16
F8 = mybir.dt.float8e4
F32 = mybir.dt.float32
import os
BD_DT = F8 if os.environ.get("BD_FP8", "0") == "1" else BF
AX = mybir.AxisListType
AF = mybir.ActivationFunctionType

N_CORES = 8
B_FULL, IC, OC, OD, ID = 256, 1152, 10, 16, 8
B_LOC = B_FULL // N_CORES          # 32
NIB = IC // 16                     # 72 i-blocks of 16
NBB = B_LOC // 8                   # 4 b-blocks of 8
F = OC * OD                        # 160

# agreement j-slices: DVE gets 3 slices, Pool one (rate-balanced 79/21)
AGR_SLICES = [(0, 19), (19, 38), (38, 57), (57, 72)]


def _squash(nc, smp, ps, scale, vout):
    """vout = squash(scale * ps) with ps an [P, 160] psum slab (f32).

    squash(s) = (n^2/(1+n^2)) * s/(n + 1e-8),  n = ||s||_2 over d.
    The 1e-8 is dropped (n >= 1e-3 in routing; rel err < 1e-5).
    """
    P = ps.shape[0]
    sq = smp.tile([P, F], F32, tag="sq")
    nc.scalar.activation(sq[:], ps[:], AF.Square, scale=float(scale))
    n2 = smp.tile([P, OC], F32, tag="n2")
    nc.vector.tensor_reduce(
        n2[:], sq[:].rearrange("p (o d) -> p o d", d=OD), axis=AX.X,
        op=mybir.AluOpType.add)
    n1 = smp.tile([P, OC], F32, tag="n1")
    nc.vector.tensor_scalar_add(n1[:], n2[:], 1.0)
    sn = smp.tile([P, OC], F32, tag="sn")
    nc.scalar.sqrt(sn[:], n2[:])
    t1 = smp.tile([P, OC], F32, tag="t1")
    nc.vector.tensor_mul(t1[:], n1[:], sn[:])
    r1 = smp.tile([P, OC], F32, tag="r1")
    nc.vector.reciprocal(r1[:], t1[:])
    f1 = smp.tile([P, OC], F32, tag="f1")
    nc.vector.tensor_mul(f1[:], n2[:], r1[:])
    if scale != 1.0:
        nc.vector.tensor_scalar_mul(f1[:], f1[:], float(scale))
    nc.vector.tensor_mul(
        vout[:].rearrange("p (o d) -> p o d", d=OD),
        ps[:].rearrange("p (o d) -> p o d", d=OD),
        f1[:].unsqueeze(-1).broadcast_to((P, OC, OD)))


def _split_multiwait(nc):
    """Walrus encodes at most ONE semaphore wait per engine/DMA instruction.
    Hoist excess waits onto same-engine NoOps placed directly before the
    instruction.  HWDGE DMAs can't be gated that way - assert instead."""
    for fn in nc.m.functions:
        for bb in fn.blocks:
            out = []
            k = 0
            for ins in bb.instructions:
                si = ins.sync_info
                waits = list(si.on_wait) if si is not None and si.on_wait else []
                limit = 1
                if ins.opcode == "DMACopy":
                    q = str(getattr(ins, "queue", "") or "")
                    if "HW" in q and len(waits) > 1:
                        raise AssertionError(
                            f"HWDGE DMA {ins.name} has {len(waits)} waits: {ins}")
                if len(waits) > limit:
                    for w in waits[:-limit]:
                        nop = mybir.InstNoOp(name=f"{ins.name}-wn{k}", ins=[], outs=[])
                        k += 1
                        nop.engine = ins.engine
                        nop.sync_info = mybir.SyncInfo(on_wait=[w], on_update=[])
                        out.append(nop)
                    ins.sync_info = mybir.SyncInfo(
                        on_wait=waits[-limit:],
                        on_update=list(si.on_update) if si.on_update else [])
                out.append(ins)
            bb.instructions = out


def build_program(split_waits=True):
    nc = bass.Bass()
    bd_d = nc.declare_dram_parameter("bd", [8, 128, 9, NBB, 128], BD_DT, isOutput=False)
    xt_d = nc.declare_dram_parameter("xt", [128, NIB, B_LOC], BF, isOutput=False)
    ws_d = nc.declare_dram_parameter("ws", [128, NIB, F], BF, isOutput=False)
    msk_d = nc.declare_dram_parameter("msk", [80, F], BF, isOutput=False)
    o80_d = nc.declare_dram_parameter("o80", [80, 8], BF, isOutput=False)
    sel_d = nc.declare_dram_parameter("sel", [8, 128], BF, isOutput=False)
    selb_d = nc.declare_dram_parameter("selb", [B_LOC, NBB, 128], BF, isOutput=False)
    mk80_d = nc.declare_dram_parameter("mk80", [128, 80], BF, isOutput=False)
    out_d = nc.declare_dram_parameter("out", [B_LOC, F], F32, isOutput=True)

    with ExitStack() as ctx:
        tc = ctx.enter_context(tile.TileContext(nc))
        st = ctx.enter_context(tc.tile_pool(name="st", bufs=1))
        bdp = ctx.enter_context(tc.tile_pool(name="bdp", bufs=2))
        y2p = ctx.enter_context(tc.tile_pool(name="y2p", bufs=1))
        tsp = ctx.enter_context(tc.tile_pool(name="tsp", bufs=1))
        mkp = ctx.enter_context(tc.tile_pool(name="mkp", bufs=2))
        cnp = ctx.enter_context(tc.tile_pool(name="cnp", bufs=1))
        cbp = ctx.enter_context(tc.tile_pool(name="cbp", bufs=1))
        vxp = ctx.enter_context(tc.tile_pool(name="vxp", bufs=2))
        smp = ctx.enter_context(tc.tile_pool(name="smp", bufs=4))
        pbig = ctx.enter_context(tc.tile_pool(name="pbig", bufs=3, space="PSUM"))
        ps1p = ctx.enter_context(tc.tile_pool(name="ps1p", bufs=2, space="PSUM"))
        ps0p = ctx.enter_context(tc.tile_pool(name="ps0p", bufs=1, space="PSUM"))
        pvxp = ctx.enter_context(tc.tile_pool(name="pvxp", bufs=1, space="PSUM"))
        psvp = ctx.enter_context(tc.tile_pool(name="psvp", bufs=1, space="PSUM"))

        # --- persistent tiles ---
        u_hat = st.tile([128, NIB, NBB, F], BF, tag="u_hat")
        ws_sb = st.tile([128, NIB, F], BF, tag="ws_sb")
        xt_sb = st.tile([128, NIB, B_LOC], BF, tag="xt_sb")
        blg = st.tile([128, NBB, NIB, OC], BF, tag="blg")
        c_sb = st.tile([128, NBB, NIB, OC], BF, tag="c_sb")
        msk_sb = st.tile([80, F], BF, tag="msk_sb")
        o80_sb = st.tile([80, 8], BF, tag="o80_sb")
        sel_sb = st.tile([8, 128], BF, tag="sel_sb")
        selb_sb = st.tile([B_LOC, NBB, 128], BF, tag="selb_sb")
        mk80_sb = st.tile([128, 80], BF, tag="mk80_sb")
        v8 = [st.tile([8, F], BF, tag=f"v8_{i}", name=f"v8_{i}") for i in range(NBB)]
        vx4 = [st.tile([128, F], BF, tag=f"vx{i}", name=f"vx{i}") for i in range(NBB)]
        vall = st.tile([B_LOC, F], BF, tag="vall")
        og = st.tile([8, NBB, F], F32, tag="og")
        sv_sb = st.tile([8, NBB, F], F32, tag="sv_sb")

        # --- input loads; pipe order xt, ws0, bd0, ws1-3 so both the
        # iter-0 GEMM chain and build-e0 start as early as possible ---
        nc.sync.dma_start(out=xt_sb[:], in_=xt_d[:])
        nc.scalar.dma_start(out=ws_sb[:, 0:18, :], in_=ws_d[:, 0:18, :])
        bdt0 = bdp.tile([128, 9, NBB, 128], BD_DT, tag="bdt")
        nc.sync.dma_start(out=bdt0[:], in_=bd_d[0])
        for wc in range(1, 4):
            nc.scalar.dma_start(
                out=ws_sb[:, wc * 18:(wc + 1) * 18, :],
                in_=ws_d[:, wc * 18:(wc + 1) * 18, :])
        nc.sync.dma_start(out=selb_sb[:], in_=selb_d[:])
        nc.sync.dma_start(out=sel_sb[:], in_=sel_d[:])
        nc.scalar.dma_start(out=msk_sb[:], in_=msk_d[:])
        nc.scalar.dma_start(out=o80_sb[:], in_=o80_d[:])
        nc.sync.dma_start(out=mk80_sb[:], in_=mk80_d[:])

        # --- iter-0 stage 1: s0 = 0.1*sum_i u_hat as one GEMM chain over the
        # full (i,k) contraction: xt[128,(iblk),32] x ws[128,(iblk),160] ---
        ps0 = ps0p.tile([B_LOC, F], F32, tag="ps0", name="ps0")
        for j in range(NIB):
            nc.tensor.matmul(
                ps0[:], lhsT=xt_sb[:, j, :], rhs=ws_sb[:, j, :],
                start=(j == 0), stop=(j == NIB - 1))

        def load_vx(bblk, first):
            pvx = pvxp.tile([128, F], F32, tag="pvx")
            if first:
                nc.tensor.matmul(
                    pvx[:], lhsT=selb_sb[:, bblk, :], rhs=vall[:],
                    start=True, stop=True)
            else:
                nc.tensor.matmul(
                    pvx[:], lhsT=sel_sb[:], rhs=v8[bblk][:],
                    start=True, stop=True)
            nc.scalar.copy(vx4[bblk][:], pvx[:])

        def agreement_slice(bblk, j0, j1, first, eng):
            """blg[:, bblk, j0:j1, :] (+)= sum_d u_hat * v for a j-slice."""
            w = j1 - j0
            pool = eng is nc.gpsimd
            y2 = y2p.tile([128, w, F], BF, tag="y2p3" if pool else "y2")
            eng.tensor_mul(
                y2[:], u_hat[:, j0:j1, bblk, :],
                vx4[bblk][:].unsqueeze(1).broadcast_to((128, w, F)))
            # in-place pairwise-add tree over d (fp16, 2x mode)
            y2v = y2[:].rearrange("p j (o d) -> p j o d", d=OD)
            eng.tensor_add(
                y2v[:, :, :, 0:8], y2v[:, :, :, 0:8], y2v[:, :, :, 8:16])
            eng.tensor_add(
                y2v[:, :, :, 0:4], y2v[:, :, :, 0:4], y2v[:, :, :, 4:8])
            eng.tensor_add(
                y2v[:, :, :, 0:2], y2v[:, :, :, 0:2], y2v[:, :, :, 2:4])
            dst = blg[:, bblk, j0:j1, :]
            if first:
                eng.tensor_add(dst, y2v[:, :, :, 0], y2v[:, :, :, 1])
            else:
                ts = tsp.tile([128, w, OC], BF, tag="tsp3" if pool else "ts")
                eng.tensor_add(ts[:], y2v[:, :, :, 0], y2v[:, :, :, 1])
                eng.tensor_add(dst, dst, ts[:])


        # --- u_hat build: one matmul per (iblk, bblk), K=128=(ii,k), N=160;
        # PSUM drained to SBUF across ACT/DVE/Pool.  Iter-0 squash runs first
        # so each e-chunk's agreement slices interleave with the build. ---
        _squash(nc, smp, ps0[:], 0.1, vall)
        for bblk in range(NBB):
            load_vx(bblk, first=True)
        drain_seq = [nc.scalar, nc.scalar, nc.scalar, nc.scalar, nc.scalar,
                     nc.scalar, nc.scalar, nc.scalar, nc.scalar, nc.vector]
        ndrain = 0
        nagr = 0
        for e in range(8):
            if e == 0:
                bdt = bdt0
            else:
                bdt = bdp.tile([128, 9, NBB, 128], BD_DT, tag="bdt")
                nc.gpsimd.dma_start(out=bdt[:], in_=bd_d[e])
            for j in range(9):
                iblk = e * 9 + j
                for h in range(2):
                    ps = pbig.tile([128, 2, F], F32, tag="pbig")
                    for bb in range(2):
                        nc.tensor.matmul(
                            ps[:, bb, :], lhsT=bdt[:, j, h * 2 + bb, :],
                            rhs=ws_sb[:, iblk, :], start=True, stop=True)
                    eng = drain_seq[ndrain % len(drain_seq)]
                    ndrain += 1
                    if eng is nc.scalar:
                        eng.copy(u_hat[:, iblk, h * 2:(h + 1) * 2, :], ps[:])
                    else:
                        eng.tensor_copy(u_hat[:, iblk, h * 2:(h + 1) * 2, :], ps[:])
            # iter-0 agreement for this e's 9 j-columns, all b-blocks
            for bblk in range(NBB):
                eng = nc.gpsimd if (bblk >= 2 and (e + bblk) % 2 == 0) else nc.vector
                nagr += 1
                agreement_slice(bblk, e * 9, e * 9 + 9, True, eng)

        # --- iters 1, 2: software-pipelined across bblks and iterations.
        # softmax+cbt feed the PE stage-1 stream; each bblk's iter-2 chain
        # starts as soon as its iter-1 agreement lands. ---
        def softmax_cbt(it, bblk):
            nc.scalar.activation(
                c_sb[:, bblk, :, :], blg[:, bblk, :, :], AF.Exp)
            sm = smp.tile([128, NIB], F32, tag="sm")
            nc.vector.tensor_reduce(
                sm[:], c_sb[:, bblk, :, :], axis=AX.X, op=mybir.AluOpType.add)
            rr = smp.tile([128, NIB], BF, tag="rr")
            with nc.allow_low_precision(reason="softmax denom recip in fp16"):
                nc.vector.reciprocal(rr[:], sm[:])
            cn = cnp.tile([128, NIB, OC], BF, tag="cn")
            nc.vector.tensor_mul(
                cn[:], c_sb[:, bblk, :, :],
                rr[:].unsqueeze(-1).broadcast_to((128, NIB, OC)))
            eng = nc.vector
            cbt = cbp.tile([128, NIB, 80], BF, tag=f"cbt{bblk % 2}")
            eng.tensor_mul(
                cbt[:],
                cn[:].unsqueeze(2).broadcast_to((128, NIB, 8, OC)),
                mk80_sb[:].rearrange("p (b o) -> p b o", o=OC).unsqueeze(1)
                .broadcast_to((128, NIB, 8, OC)))
            ps1 = ps1p.tile([80, F], F32, tag="ps1", name=f"ps1_{it}_{bblk}")
            for j in range(NIB):
                nc.tensor.matmul(
                    ps1[:], lhsT=cbt[:, j, :],
                    rhs=u_hat[:, j, bblk, :],
                    start=(j == 0), stop=(j == NIB - 1))
            return ps1

        def extract_s(ps1):
            mskd = mkp.tile([80, F], BF, tag="mskd")
            nc.vector.tensor_mul(mskd[:], ps1[:], msk_sb[:])
            psv = psvp.tile([8, F], F32, tag="psv")
            nc.tensor.matmul(
                psv[:], lhsT=o80_sb[:], rhs=mskd[:], start=True, stop=True)
            return psv

        for it in (1, 2):
            ps1s = [softmax_cbt(it, bb) for bb in range(NBB)]
            for bblk in range(NBB):
                psv = extract_s(ps1s[bblk])
                if it == 1:
                    _squash(nc, smp, psv[:], 1.0, v8[bblk])
                    load_vx(bblk, first=False)
                    agreement_slice(bblk, 0, 25, False, nc.vector)
                    agreement_slice(bblk, 25, 50, False, nc.vector)
                    agreement_slice(bblk, 50, 72, False, nc.gpsimd)
                else:
                    nc.scalar.copy(sv_sb[:, bblk, :], psv[:])
        # batched final squash over all 4 b-blocks: 8 ops instead of 32,
        # cutting the latency-bound serial tail
        sq4 = st.tile([8, NBB, F], F32, tag="sq4")
        nc.vector.tensor_mul(sq4[:], sv_sb[:], sv_sb[:])
        n24 = st.tile([8, NBB, OC], F32, tag="n24")
        nc.vector.tensor_reduce(
            n24[:], sq4[:].rearrange("p b (o d) -> p b o d", d=OD), axis=AX.X,
            op=mybir.AluOpType.add)
        n14 = st.tile([8, NBB, OC], F32, tag="n14")
        nc.vector.tensor_scalar_add(n14[:], n24[:], 1.0)
        sn4 = st.tile([8, NBB, OC], F32, tag="sn4")
        nc.scalar.sqrt(sn4[:], n24[:])
        t14 = st.tile([8, NBB, OC], F32, tag="t14")
        nc.vector.tensor_mul(t14[:], n14[:], sn4[:])
        r14 = st.tile([8, NBB, OC], F32, tag="r14")
        nc.vector.reciprocal(r14[:], t14[:])
        f14 = st.tile([8, NBB, OC], F32, tag="f14")
        nc.vector.tensor_mul(f14[:], n24[:], r14[:])
        nc.vector.tensor_mul(
            og[:].rearrange("p b (o d) -> p b o d", d=OD),
            sv_sb[:].rearrange("p b (o d) -> p b o d", d=OD),
            f14[:].unsqueeze(-1).broadcast_to((8, NBB, OC, OD)))
        nc.gpsimd.dma_start(
            out=out_d[:].rearrange("(bb bp) f -> bp bb f", bp=8), in_=og[:])

    if split_waits:
        _split_multiwait(nc)
    return nc


def _host_inputs(x, W):
    """Per-core input maps from full x [256,1152,8] f32, W [1,1152,10,16,8] f32."""
    bf = np.float16
    f8 = mybir.dt.np(BD_DT)
    W0 = np.asarray(W[0], dtype=np.float32)
    # ws[q=(ii,k), j, (o,d)] = W[j*16+ii, o, d, k]
    ws = np.ascontiguousarray(
        W0.reshape(NIB, 16, OC, OD, ID).transpose(1, 4, 0, 2, 3)
        .reshape(128, NIB, F)).astype(bf)
    msk = np.zeros((80, F), dtype=bf)
    for bpp in range(8):
        for o in range(OC):
            msk[bpp * 10 + o, o * OD:(o + 1) * OD] = 1.0
    o80 = np.zeros((80, 8), dtype=bf)
    for p in range(80):
        o80[p, p // 10] = 1.0
    sel = np.zeros((8, 128), dtype=bf)
    for p in range(128):
        sel[p // 16, p] = 1.0
    mk80 = np.zeros((128, 80), dtype=bf)
    for p in range(128):
        mk80[p, (p // 16) * 10:(p // 16) * 10 + 10] = 1.0
    selb = np.zeros((B_LOC, NBB, 128), dtype=bf)
    for bb in range(NBB):
        for p in range(128):
            selb[bb * 8 + p // 16, bb, p] = 1.0

    in_maps = []
    for c in range(N_CORES):
        xc = np.asarray(x[c * B_LOC:(c + 1) * B_LOC], dtype=np.float32)
        # bd[e, q=(ii,k), j, bb, m=(bp,ii')] = x[bb*8+bp, (e*9+j)*16+ii, k] iff ii'==ii
        r = xc.reshape(NBB, 8, 8, 9, 16, ID)          # [bb, bp, e, j, ii, k]
        bd6 = np.zeros((8, 16, ID, 9, NBB, 8, 16), dtype=np.float32)
        for ii in range(16):
            bd6[:, ii, :, :, :, :, ii] = r[:, :, :, :, ii, :].transpose(2, 4, 3, 0, 1)
        bd = np.ascontiguousarray(bd6.reshape(8, 128, 9, NBB, 128)).astype(f8)
        # xt[q=(ii,k), iblk, b] = x[b, iblk*16+ii, k]
        xt = np.ascontiguousarray(
            xc.reshape(B_LOC, NIB, 16, ID).transpose(2, 3, 1, 0)
            .reshape(128, NIB, B_LOC)).astype(bf)
        in_maps.append(
            {"bd": bd, "xt": xt, "ws": ws, "msk": msk, "o80": o80, "sel": sel,
             "mk80": mk80, "selb": selb})
    return in_maps


_NC_CACHE = {}


def kernel(x, W):
    from concourse.bass_utils import run_bass_kernel_spmd

    if "nc" not in _NC_CACHE:
        _NC_CACHE["nc"] = build_program()
    nc = _NC_CACHE["nc"]
    in_maps = _host_inputs(x, W)
    res = run_bass_kernel_spmd(nc, in_maps, core_ids=list(range(N_CORES)))
    out = np.concatenate([r["out"] for r in res.results], axis=0)
    return out.reshape(B_FULL, OC, OD).astype(np.float32)


if __name__ == "__main__":
    nc = build_program()
    print("program built ok")


# revision 20
# speedup vs baseline: 1.0919x; 1.0010x over previous
"""Trainium2 Bass kernel for a CapsuleNet dynamic-routing layer (v2).

Math (per batch element b):
    u_hat[b,i,o,d] = sum_k W[i,o,d,k] * x[b,i,k]      # B=256, IC=1152, OC=10, OD=16, ID=8
    b_log = 0
    for it in 0..2:
        c = softmax(b_log, axis=o)
        s[b,o,d] = sum_i c[b,i,o] * u_hat[b,i,o,d]
        v = squash(s)
        if it < 2: b_log += sum_d u_hat * v

Sharding: data-parallel over B across 8 cores (32 local rows), W replicated.

v2 changes vs v1 (cost-model driven):
  - W loaded once (one HWDGE DMA) instead of 16 Pool-queue DMAs.
  - iter-0 s = 0.1*sum_i u_hat computed as ONE accumulating GEMM chain
    (72 matmuls, M=32) instead of 288 M=8 matmuls.
  - bd (block-diag x) shipped as fp8e4: halves the dominant HBM load.
  - c block-diag scatter moved from Pool-queue (994ns prep each) to
    SP HWDGE queues.
  - PSUM drains split ACT/DVE/Pool; agreement split DVE ~79% / Pool ~21%
    (DVE 2x mode = 0.52 ns/col vs Pool 1.98).
  - blg stored o-last so agreement writes are packed (2x); softmax exp
    transposes to j-last via the ACT engine (free there) so the scatter
    DMA sees 144B contiguous runs.
"""

import sys

sys.path.insert(0, "/opt/trn_rl_repo")

from contextlib import ExitStack

import numpy as np

import concourse.bass as bass
import concourse.tile as tile
from concourse import mybir

BF = mybir.dt.float16
F8 = mybir.dt.float8e4
F32 = mybir.dt.float32
import os
BD_DT = F8 if os.environ.get("BD_FP8", "0") == "1" else BF
AX = mybir.AxisListType
AF = mybir.ActivationFunctionType

N_CORES = 8
B_FULL, IC, OC, OD, ID = 256, 1152, 10, 16, 8
B_LOC = B_FULL // N_CORES          # 32
NIB = IC // 16                     # 72 i-blocks of 16
NBB = B_LOC // 8                   # 4 b-blocks of 8
F = OC * OD                        # 160

# agreement j-slices: DVE gets 3 slices, Pool one (rate-balanced 79/21)
AGR_SLICES = [(0, 19), (19, 38), (38, 57), (57, 72)]


def _squash(nc, smp, ps, scale, vout):
    """vout = squash(scale * ps) with ps an [P, 160] psum slab (f32).

    squash(s) = (n^2/(1+n^2)) * s/(n + 1e-8),  n = ||s||_2 over d.
    The 1e-8 is dropped (n >= 1e-3 in routing; rel err < 1e-5).
    """
    P = ps.shape[0]
    sq = smp.tile([P, F], F32, tag="sq")
    nc.scalar.activation(sq[:], ps[:], AF.Square, scale=float(scale))
    n2 = smp.tile([P, OC], F32, tag="n2")
    nc.vector.tensor_reduce(
        n2[:], sq[:].rearrange("p (o d) -> p o d", d=OD), axis=AX.X,
        op=mybir.AluOpType.add)
    n1 = smp.tile([P, OC], F32, tag="n1")
    nc.vector.tensor_scalar_add(n1[:], n2[:], 1.0)
    sn = smp.tile([P, OC], F32, tag="sn")
    nc.scalar.sqrt(sn[:], n2[:])
    t1 = smp.tile([P, OC], F32, tag="t1")
    nc.vector.tensor_mul(t1[:], n1[:], sn[:])
    r1 = smp.tile([P, OC], F32, tag="r1")
    nc.vector.reciprocal(r1[:], t1[:])
    f1 = smp.tile([P, OC], F32, tag="f1")
    nc.vector.tensor_mul(f1[:], n2[:], r1[:])
    if scale != 1.0:
        nc.vector.tensor_scalar_mul(f1[:], f1[:], float(scale))
    nc.vector.tensor_mul(
        vout[:].rearrange("p (o d) -> p o d", d=OD),
        ps[:].rearrange("p (o d) -> p o d", d=OD),
        f1[:].unsqueeze(-1).broadcast_to((P, OC, OD)))


def _split_multiwait(nc):
    """Walrus encodes at most ONE semaphore wait per engine/DMA instruction.
    Hoist excess waits onto same-engine NoOps placed directly before the
    instruction.  HWDGE DMAs can't be gated that way - assert instead."""
    for fn in nc.m.functions:
        for bb in fn.blocks:
            out = []
            k = 0
            for ins in bb.instructions:
                si = ins.sync_info
                waits = list(si.on_wait) if si is not None and si.on_wait else []
                limit = 1
                if ins.opcode == "DMACopy":
                    q = str(getattr(ins, "queue", "") or "")
                    if "HW" in q and len(waits) > 1:
                        raise AssertionError(
                            f"HWDGE DMA {ins.name} has {len(waits)} waits: {ins}")
                if len(waits) > limit:
                    for w in waits[:-limit]:
                        nop = mybir.InstNoOp(name=f"{ins.name}-wn{k}", ins=[], outs=[])
                        k += 1
                        nop.engine = ins.engine
                        nop.sync_info = mybir.SyncInfo(on_wait=[w], on_update=[])
                        out.append(nop)
                    ins.sync_info = mybir.SyncInfo(
                        on_wait=waits[-limit:],
                        on_update=list(si.on_update) if si.on_update else [])
                out.append(ins)
            bb.instructions = out


def build_program(split_waits=True):
    nc = bass.Bass()
    bd_d = nc.declare_dram_parameter("bd", [8, 128, 9, NBB, 128], BD_DT, isOutput=False)
    xt_d = nc.declare_dram_parameter("xt", [128, NIB, B_LOC], BF, isOutput=False)
    ws_d = nc.declare_dram_parameter("ws", [128, NIB, F], BF, isOutput=False)
    msk_d = nc.declare_dram_parameter("msk", [80, F], BF, isOutput=False)
    o80_d = nc.declare_dram_parameter("o80", [80, 8], BF, isOutput=False)
    sel_d = nc.declare_dram_parameter("sel", [8, 128], BF, isOutput=False)
    selb_d = nc.declare_dram_parameter("selb", [B_LOC, NBB, 128], BF, isOutput=False)
    mk80_d = nc.declare_dram_parameter("mk80", [128, 80], BF, isOutput=False)
    out_d = nc.declare_dram_parameter("out", [B_LOC, F], F32, isOutput=True)

    with ExitStack() as ctx:
        tc = ctx.enter_context(tile.TileContext(nc))
        st = ctx.enter_context(tc.tile_pool(name="st", bufs=1))
        bdp = ctx.enter_context(tc.tile_pool(name="bdp", bufs=2))
        y2p = ctx.enter_context(tc.tile_pool(name="y2p", bufs=1))
        tsp = ctx.enter_context(tc.tile_pool(name="tsp", bufs=1))
        mkp = ctx.enter_context(tc.tile_pool(name="mkp", bufs=2))
        cnp = ctx.enter_context(tc.tile_pool(name="cnp", bufs=1))
        cbp = ctx.enter_context(tc.tile_pool(name="cbp", bufs=1))
        vxp = ctx.enter_context(tc.tile_pool(name="vxp", bufs=2))
        smp = ctx.enter_context(tc.tile_pool(name="smp", bufs=4))
        pbig = ctx.enter_context(tc.tile_pool(name="pbig", bufs=3, space="PSUM"))
        ps1p = ctx.enter_context(tc.tile_pool(name="ps1p", bufs=2, space="PSUM"))
        ps0p = ctx.enter_context(tc.tile_pool(name="ps0p", bufs=1, space="PSUM"))
        pvxp = ctx.enter_context(tc.tile_pool(name="pvxp", bufs=1, space="PSUM"))
        psvp = ctx.enter_context(tc.tile_pool(name="psvp", bufs=1, space="PSUM"))

        # --- persistent tiles ---
        u_hat = st.tile([128, NIB, NBB, F], BF, tag="u_hat")
        ws_sb = st.tile([128, NIB, F], BF, tag="ws_sb")
        xt_sb = st.tile([128, NIB, B_LOC], BF, tag="xt_sb")
        blg = st.tile([128, NBB, NIB, OC], BF, tag="blg")
        c_sb = st.tile([128, NBB, NIB, OC], BF, tag="c_sb")
        msk_sb = st.tile([80, F], BF, tag="msk_sb")
        o80_sb = st.tile([80, 8], BF, tag="o80_sb")
        sel_sb = st.tile([8, 128], BF, tag="sel_sb")
        selb_sb = st.tile([B_LOC, NBB, 128], BF, tag="selb_sb")
        mk80_sb = st.tile([128, 80], BF, tag="mk80_sb")
        v8 = [st.tile([8, F], BF, tag=f"v8_{i}", name=f"v8_{i}") for i in range(NBB)]
        vx4 = [st.tile([128, F], BF, tag=f"vx{i}", name=f"vx{i}") for i in range(NBB)]
        vall = st.tile([B_LOC, F], BF, tag="vall")
        vb8 = st.tile([8, NBB, F], BF, tag="vb8")
        og = st.tile([8, NBB, F], F32, tag="og")
        sv_sb = st.tile([8, NBB, F], F32, tag="sv_sb")

        # --- input loads; pipe order xt, ws0, bd0, ws1-3 so both the
        # iter-0 GEMM chain and build-e0 start as early as possible ---
        nc.sync.dma_start(out=xt_sb[:], in_=xt_d[:])
        nc.scalar.dma_start(out=ws_sb[:, 0:18, :], in_=ws_d[:, 0:18, :])
        bdt0 = bdp.tile([128, 9, NBB, 128], BD_DT, tag="bdt")
        nc.sync.dma_start(out=bdt0[:], in_=bd_d[0])
        for wc in range(1, 4):
            nc.scalar.dma_start(
                out=ws_sb[:, wc * 18:(wc + 1) * 18, :],
                in_=ws_d[:, wc * 18:(wc + 1) * 18, :])
        nc.sync.dma_start(out=selb_sb[:], in_=selb_d[:])
        nc.sync.dma_start(out=sel_sb[:], in_=sel_d[:])
        nc.scalar.dma_start(out=msk_sb[:], in_=msk_d[:])
        nc.scalar.dma_start(out=o80_sb[:], in_=o80_d[:])
        nc.sync.dma_start(out=mk80_sb[:], in_=mk80_d[:])

        # --- iter-0 stage 1: s0 = 0.1*sum_i u_hat as one GEMM chain over the
        # full (i,k) contraction: xt[128,(iblk),32] x ws[128,(iblk),160] ---
        ps0 = ps0p.tile([B_LOC, F], F32, tag="ps0", name="ps0")
        for j in range(NIB):
            nc.tensor.matmul(
                ps0[:], lhsT=xt_sb[:, j, :], rhs=ws_sb[:, j, :],
                start=(j == 0), stop=(j == NIB - 1))

        def load_vx(bblk, first):
            pvx = pvxp.tile([128, F], F32, tag="pvx")
            if first:
                nc.tensor.matmul(
                    pvx[:], lhsT=selb_sb[:, bblk, :], rhs=vall[:],
                    start=True, stop=True)
            else:
                nc.tensor.matmul(
                    pvx[:], lhsT=sel_sb[:], rhs=v8[bblk][:],
                    start=True, stop=True)
            nc.scalar.copy(vx4[bblk][:], pvx[:])

        def agreement_slice(bblk, j0, j1, first, eng):
            """blg[:, bblk, j0:j1, :] (+)= sum_d u_hat * v for a j-slice."""
            w = j1 - j0
            pool = eng is nc.gpsimd
            y2 = y2p.tile([128, w, F], BF, tag="y2p3" if pool else "y2")
            eng.tensor_mul(
                y2[:], u_hat[:, j0:j1, bblk, :],
                vx4[bblk][:].unsqueeze(1).broadcast_to((128, w, F)))
            # in-place pairwise-add tree over d (fp16, 2x mode)
            y2v = y2[:].rearrange("p j (o d) -> p j o d", d=OD)
            eng.tensor_add(
                y2v[:, :, :, 0:8], y2v[:, :, :, 0:8], y2v[:, :, :, 8:16])
            eng.tensor_add(
                y2v[:, :, :, 0:4], y2v[:, :, :, 0:4], y2v[:, :, :, 4:8])
            eng.tensor_add(
                y2v[:, :, :, 0:2], y2v[:, :, :, 0:2], y2v[:, :, :, 2:4])
            dst = blg[:, bblk, j0:j1, :]
            if first:
                eng.tensor_add(dst, y2v[:, :, :, 0], y2v[:, :, :, 1])
            else:
                ts = tsp.tile([128, w, OC], BF, tag="tsp3" if pool else "ts")
                eng.tensor_add(ts[:], y2v[:, :, :, 0], y2v[:, :, :, 1])
                eng.tensor_add(dst, dst, ts[:])


        # --- u_hat build: one matmul per (iblk, bblk), K=128=(ii,k), N=160;
        # PSUM drained to SBUF across ACT/DVE/Pool.  Iter-0 squash runs first
        # so each e-chunk's agreement slices interleave with the build. ---
        _squash(nc, smp, ps0[:], 0.1, vall)
        for bblk in range(NBB):
            load_vx(bblk, first=True)
        drain_seq = [nc.scalar, nc.scalar, nc.scalar, nc.scalar, nc.scalar,
                     nc.scalar, nc.scalar, nc.scalar, nc.scalar, nc.vector]
        ndrain = 0
        nagr = 0
        for e in range(8):
            if e == 0:
                bdt = bdt0
            else:
                bdt = bdp.tile([128, 9, NBB, 128], BD_DT, tag="bdt")
                nc.gpsimd.dma_start(out=bdt[:], in_=bd_d[e])
            for j in range(9):
                iblk = e * 9 + j
                for h in range(2):
                    ps = pbig.tile([128, 2, F], F32, tag="pbig")
                    for bb in range(2):
                        nc.tensor.matmul(
                            ps[:, bb, :], lhsT=bdt[:, j, h * 2 + bb, :],
                            rhs=ws_sb[:, iblk, :], start=True, stop=True)
                    eng = drain_seq[ndrain % len(drain_seq)]
                    ndrain += 1
                    if eng is nc.scalar:
                        eng.copy(u_hat[:, iblk, h * 2:(h + 1) * 2, :], ps[:])
                    else:
                        eng.tensor_copy(u_hat[:, iblk, h * 2:(h + 1) * 2, :], ps[:])
            # iter-0 agreement in 18-j sets (every 2nd e-chunk): halves the
            # per-op fixed cost on DVE vs 9-j sets
            if e % 2 == 1:
                for bblk in range(NBB):
                    eng = nc.gpsimd if bblk == (e // 2) % 4 else nc.vector
                    nagr += 1
                    agreement_slice(bblk, (e - 1) * 9, (e + 1) * 9, True, eng)

        # --- iters 1, 2: software-pipelined across bblks and iterations.
        # softmax+cbt feed the PE stage-1 stream; each bblk's iter-2 chain
        # starts as soon as its iter-1 agreement lands. ---
        def softmax_cbt(it, bblk):
            nc.scalar.activation(
                c_sb[:, bblk, :, :], blg[:, bblk, :, :], AF.Exp)
            sm = smp.tile([128, NIB], F32, tag="sm")
            nc.vector.tensor_reduce(
                sm[:], c_sb[:, bblk, :, :], axis=AX.X, op=mybir.AluOpType.add)
            rr = smp.tile([128, NIB], BF, tag="rr")
            with nc.allow_low_precision(reason="softmax denom recip in fp16"):
                nc.vector.reciprocal(rr[:], sm[:])
            cn = cnp.tile([128, NIB, OC], BF, tag="cn")
            nc.vector.tensor_mul(
                cn[:], c_sb[:, bblk, :, :],
                rr[:].unsqueeze(-1).broadcast_to((128, NIB, OC)))
            eng = nc.vector
            cbt = cbp.tile([128, NIB, 80], BF, tag=f"cbt{bblk % 2}")
            eng.tensor_mul(
                cbt[:],
                cn[:].unsqueeze(2).broadcast_to((128, NIB, 8, OC)),
                mk80_sb[:].rearrange("p (b o) -> p b o", o=OC).unsqueeze(1)
                .broadcast_to((128, NIB, 8, OC)))
            ps1 = ps1p.tile([80, F], F32, tag="ps1", name=f"ps1_{it}_{bblk}")
            for j in range(NIB):
                nc.tensor.matmul(
                    ps1[:], lhsT=cbt[:, j, :],
                    rhs=u_hat[:, j, bblk, :],
                    start=(j == 0), stop=(j == NIB - 1))
            return ps1

        def extract_s(ps1):
            mskd = mkp.tile([80, F], BF, tag="mskd")
            nc.vector.tensor_mul(mskd[:], ps1[:], msk_sb[:])
            psv = psvp.tile([8, F], F32, tag="psv")
            nc.tensor.matmul(
                psv[:], lhsT=o80_sb[:], rhs=mskd[:], start=True, stop=True)
            return psv

        def batched_squash(dst):
            """dst[8, NBB, F] = squash(sv_sb) over all 4 b-blocks at once:
            8 ops instead of 32, cutting the latency-bound serial chain."""
            sq4 = st.tile([8, NBB, F], F32, tag="sq4")
            nc.vector.tensor_mul(sq4[:], sv_sb[:], sv_sb[:])
            n24 = st.tile([8, NBB, OC], F32, tag="n24")
            nc.vector.tensor_reduce(
                n24[:], sq4[:].rearrange("p b (o d) -> p b o d", d=OD),
                axis=AX.X, op=mybir.AluOpType.add)
            n14 = st.tile([8, NBB, OC], F32, tag="n14")
            nc.vector.tensor_scalar_add(n14[:], n24[:], 1.0)
            sn4 = st.tile([8, NBB, OC], F32, tag="sn4")
            nc.scalar.sqrt(sn4[:], n24[:])
            t14 = st.tile([8, NBB, OC], F32, tag="t14")
            nc.vector.tensor_mul(t14[:], n14[:], sn4[:])
            r14 = st.tile([8, NBB, OC], F32, tag="r14")
            nc.vector.reciprocal(r14[:], t14[:])
            f14 = st.tile([8, NBB, OC], F32, tag="f14")
            nc.vector.tensor_mul(f14[:], n24[:], r14[:])
            nc.vector.tensor_mul(
                dst[:].rearrange("p b (o d) -> p b o d", d=OD),
                sv_sb[:].rearrange("p b (o d) -> p b o d", d=OD),
                f14[:].unsqueeze(-1).broadcast_to((8, NBB, OC, OD)))

        for it in (1, 2):
            ps1s = [softmax_cbt(it, bb) for bb in range(NBB)]
            for bblk in range(NBB):
                psv = extract_s(ps1s[bblk])
                if it == 1:
                    _squash(nc, smp, psv[:], 1.0, v8[bblk])
                    load_vx(bblk, first=False)
                    agreement_slice(bblk, 0, 25, False, nc.vector)
                    agreement_slice(bblk, 25, 50, False, nc.vector)
                    agreement_slice(bblk, 50, 72, False, nc.gpsimd)
                else:
                    nc.scalar.copy(sv_sb[:, bblk, :], psv[:])
            if it == 2:
                batched_squash(og)
        nc.gpsimd.dma_start(
            out=out_d[:].rearrange("(bb bp) f -> bp bb f", bp=8), in_=og[:])

    if split_waits:
        _split_multiwait(nc)
    return nc


def _host_inputs(x, W):
    """Per-core input maps from full x [256,1152,8] f32, W [1,1152,10,16,8] f32."""
    bf = np.float16
    f8 = mybir.dt.np(BD_DT)
    W0 = np.asarray(W[0], dtype=np.float32)
    # ws[q=(ii,k), j, (o,d)] = W[j*16+ii, o, d, k]
    ws = np.ascontiguousarray(
        W0.reshape(NIB, 16, OC, OD, ID).transpose(1, 4, 0, 2, 3)
        .reshape(128, NIB, F)).astype(bf)
    msk = np.zeros((80, F), dtype=bf)
    for bpp in range(8):
        for o in range(OC):
            msk[bpp * 10 + o, o * OD:(o + 1) * OD] = 1.0
    o80 = np.zeros((80, 8), dtype=bf)
    for p in range(80):
        o80[p, p // 10] = 1.0
    sel = np.zeros((8, 128), dtype=bf)
    for p in range(128):
        sel[p // 16, p] = 1.0
    mk80 = np.zeros((128, 80), dtype=bf)
    for p in range(128):
        mk80[p, (p // 16) * 10:(p // 16) * 10 + 10] = 1.0
    selb = np.zeros((B_LOC, NBB, 128), dtype=bf)
    for bb in range(NBB):
        for p in range(128):
            selb[bb * 8 + p // 16, bb, p] = 1.0

    in_maps = []
    for c in range(N_CORES):
        xc = np.asarray(x[c * B_LOC:(c + 1) * B_LOC], dtype=np.float32)
        # bd[e, q=(ii,k), j, bb, m=(bp,ii')] = x[bb*8+bp, (e*9+j)*16+ii, k] iff ii'==ii
        r = xc.reshape(NBB, 8, 8, 9, 16, ID)          # [bb, bp, e, j, ii, k]
        bd6 = np.zeros((8, 16, ID, 9, NBB, 8, 16), dtype=np.float32)
        for ii in range(16):
            bd6[:, ii, :, :, :, :, ii] = r[:, :, :, :, ii, :].transpose(2, 4, 3, 0, 1)
        bd = np.ascontiguousarray(bd6.reshape(8, 128, 9, NBB, 128)).astype(f8)
        # xt[q=(ii,k), iblk, b] = x[b, iblk*16+ii, k]
        xt = np.ascontiguousarray(
            xc.reshape(B_LOC, NIB, 16, ID).transpose(2, 3, 1, 0)
            .reshape(128, NIB, B_LOC)).astype(bf)
        in_maps.append(
            {"bd": bd, "xt": xt, "ws": ws, "msk": msk, "o80": o80, "sel": sel,
             "mk80": mk80, "selb": selb})
    return in_maps


_NC_CACHE = {}


def kernel(x, W):
    from concourse.bass_utils import run_bass_kernel_spmd

    if "nc" not in _NC_CACHE:
        _NC_CACHE["nc"] = build_program()
    nc = _NC_CACHE["nc"]
    in_maps = _host_inputs(x, W)
    res = run_bass_kernel_spmd(nc, in_maps, core_ids=list(range(N_CORES)))
    out = np.concatenate([r["out"] for r in res.results], axis=0)
    return out.reshape(B_FULL, OC, OD).astype(np.float32)


if __name__ == "__main__":
    nc = build_program()
    print("program built ok")


# revision 29
# speedup vs baseline: 1.1188x; 1.0247x over previous
"""Trainium2 Bass kernel for a CapsuleNet dynamic-routing layer (v2).

Math (per batch element b):
    u_hat[b,i,o,d] = sum_k W[i,o,d,k] * x[b,i,k]      # B=256, IC=1152, OC=10, OD=16, ID=8
    b_log = 0
    for it in 0..2:
        c = softmax(b_log, axis=o)
        s[b,o,d] = sum_i c[b,i,o] * u_hat[b,i,o,d]
        v = squash(s)
        if it < 2: b_log += sum_d u_hat * v

Sharding: data-parallel over B across 8 cores (32 local rows), W replicated.

v2 changes vs v1 (cost-model driven; ~272us -> ~205us modeled):
  - W loaded once (chunked HWDGE DMAs) instead of 16 Pool-queue DMAs;
    iter-0 s = 0.1*sum_i u_hat is ONE accumulating 72-matmul GEMM (M=32)
    with a single batched squash, instead of 288 M=8 matmuls.
  - c block-diag built by a masked broadcast-multiply on DVE (2x mode)
    instead of 64 Pool-queue scatter DMAs; the mask writes the zeros so
    no memsets or persistent diag buffers are needed.
  - blg/c stored o-last so agreement writes and softmax run packed (2x).
  - PSUM drains 9:1 ACT:DVE (GPSIMD cannot touch PSUM on HW; an op may
    read at most one PSUM operand); agreement split DVE/Pool ~70/30.
  - iter-0 agreement interleaved with the u_hat build per 2 e-chunks;
    iters 1-2 run softmax+cbt for all b-blocks first so the PE stage-1
    stream overlaps the extraction/agreement wave; the final 4 squashes
    are batched into one 8-op chain; one merged SWDGE output DMA.

  NOTE: fp8 for x/W was tried and is mathematically unsound here: in
  s = sum_i c*u both signal and quantization error grow as sqrt(N), so
  the ~3% per-element fp8 error lands on the output unattenuated.
"""

import sys

sys.path.insert(0, "/opt/trn_rl_repo")

from contextlib import ExitStack

import numpy as np

import concourse.bass as bass
import concourse.tile as tile
from concourse import mybir

BF = mybir.dt.float16
F8 = mybir.dt.float8e4
F32 = mybir.dt.float32
import os
BD_DT = F8 if os.environ.get("BD_FP8", "0") == "1" else BF
AX = mybir.AxisListType
AF = mybir.ActivationFunctionType

N_CORES = 8
B_FULL, IC, OC, OD, ID = 256, 1152, 10, 16, 8
B_LOC = B_FULL // N_CORES          # 32
NIB = IC // 16                     # 72 i-blocks of 16
NBB = B_LOC // 8                   # 4 b-blocks of 8
F = OC * OD                        # 160

def _squash(nc, smp, ps, scale, vout):
    """vout = squash(scale * ps) with ps an [P, 160] psum slab (f32).

    squash(s) = (n^2/(1+n^2)) * s/(n + 1e-8),  n = ||s||_2 over d.
    The 1e-8 is dropped (n >= 1e-3 in routing; rel err < 1e-5).
    """
    P = ps.shape[0]
    sq = smp.tile([P, F], F32, tag="sq")
    nc.scalar.activation(sq[:], ps[:], AF.Square, scale=float(scale))
    n2 = smp.tile([P, OC], F32, tag="n2")
    nc.vector.tensor_reduce(
        n2[:], sq[:].rearrange("p (o d) -> p o d", d=OD), axis=AX.X,
        op=mybir.AluOpType.add)
    n1 = smp.tile([P, OC], F32, tag="n1")
    nc.vector.tensor_scalar_add(n1[:], n2[:], 1.0)
    sn = smp.tile([P, OC], F32, tag="sn")
    nc.scalar.sqrt(sn[:], n2[:])
    t1 = smp.tile([P, OC], F32, tag="t1")
    nc.vector.tensor_mul(t1[:], n1[:], sn[:])
    r1 = smp.tile([P, OC], F32, tag="r1")
    nc.vector.reciprocal(r1[:], t1[:])
    f1 = smp.tile([P, OC], F32, tag="f1")
    nc.vector.tensor_mul(f1[:], n2[:], r1[:])
    if scale != 1.0:
        nc.vector.tensor_scalar_mul(f1[:], f1[:], float(scale))
    nc.vector.tensor_mul(
        vout[:].rearrange("p (o d) -> p o d", d=OD),
        ps[:].rearrange("p (o d) -> p o d", d=OD),
        f1[:].unsqueeze(-1).broadcast_to((P, OC, OD)))


def _split_multiwait(nc):
    """Walrus encodes at most ONE semaphore wait per engine/DMA instruction.
    Hoist excess waits onto same-engine NoOps placed directly before the
    instruction.  HWDGE DMAs can't be gated that way - assert instead."""
    for fn in nc.m.functions:
        for bb in fn.blocks:
            out = []
            k = 0
            for ins in bb.instructions:
                si = ins.sync_info
                waits = list(si.on_wait) if si is not None and si.on_wait else []
                limit = 1
                if ins.opcode == "DMACopy":
                    q = str(getattr(ins, "queue", "") or "")
                    if "HW" in q and len(waits) > 1:
                        raise AssertionError(
                            f"HWDGE DMA {ins.name} has {len(waits)} waits: {ins}")
                if len(waits) > limit:
                    for w in waits[:-limit]:
                        nop = mybir.InstNoOp(name=f"{ins.name}-wn{k}", ins=[], outs=[])
                        k += 1
                        nop.engine = ins.engine
                        nop.sync_info = mybir.SyncInfo(on_wait=[w], on_update=[])
                        out.append(nop)
                    ins.sync_info = mybir.SyncInfo(
                        on_wait=waits[-limit:],
                        on_update=list(si.on_update) if si.on_update else [])
                out.append(ins)
            bb.instructions = out


def build_program(split_waits=True):
    nc = bass.Bass()
    bd_d = nc.declare_dram_parameter("bd", [8, 128, 9, NBB, 128], BD_DT, isOutput=False)
    xt_d = nc.declare_dram_parameter("xt", [128, NIB, B_LOC], BF, isOutput=False)
    ws_d = nc.declare_dram_parameter("ws", [128, NIB, F], BF, isOutput=False)
    msk_d = nc.declare_dram_parameter("msk", [80, F], BF, isOutput=False)
    o80_d = nc.declare_dram_parameter("o80", [80, 8], BF, isOutput=False)
    sel_d = nc.declare_dram_parameter("sel", [8, 128], BF, isOutput=False)
    selb_d = nc.declare_dram_parameter("selb", [B_LOC, NBB, 128], BF, isOutput=False)
    mk80_d = nc.declare_dram_parameter("mk80", [128, 80], BF, isOutput=False)
    out_d = nc.declare_dram_parameter("out", [B_LOC, F], F32, isOutput=True)

    with ExitStack() as ctx:
        tc = ctx.enter_context(tile.TileContext(nc))
        st = ctx.enter_context(tc.tile_pool(name="st", bufs=1))
        bdp = ctx.enter_context(tc.tile_pool(name="bdp", bufs=2))
        y2p = ctx.enter_context(tc.tile_pool(name="y2p", bufs=1))
        tsp = ctx.enter_context(tc.tile_pool(name="tsp", bufs=1))
        mkp = ctx.enter_context(tc.tile_pool(name="mkp", bufs=2))
        cnp = ctx.enter_context(tc.tile_pool(name="cnp", bufs=1))
        cbp = ctx.enter_context(tc.tile_pool(name="cbp", bufs=1))
        vxp = ctx.enter_context(tc.tile_pool(name="vxp", bufs=2))
        smp = ctx.enter_context(tc.tile_pool(name="smp", bufs=4))
        pbig = ctx.enter_context(tc.tile_pool(name="pbig", bufs=3, space="PSUM"))
        ps1p = ctx.enter_context(tc.tile_pool(name="ps1p", bufs=2, space="PSUM"))
        ps0p = ctx.enter_context(tc.tile_pool(name="ps0p", bufs=1, space="PSUM"))
        pvxp = ctx.enter_context(tc.tile_pool(name="pvxp", bufs=1, space="PSUM"))
        psvp = ctx.enter_context(tc.tile_pool(name="psvp", bufs=1, space="PSUM"))

        # --- persistent tiles ---
        u_hat = st.tile([128, NIB, NBB, F], BF, tag="u_hat")
        ws_sb = st.tile([128, NIB, F], BF, tag="ws_sb")
        xt_sb = st.tile([128, NIB, B_LOC], BF, tag="xt_sb")
        blg = st.tile([128, NBB, NIB, OC], BF, tag="blg")
        c_sb = st.tile([128, NBB, NIB, OC], BF, tag="c_sb")
        msk_sb = st.tile([80, F], BF, tag="msk_sb")
        o80_sb = st.tile([80, 8], BF, tag="o80_sb")
        sel_sb = st.tile([8, 128], BF, tag="sel_sb")
        selb_sb = st.tile([B_LOC, NBB, 128], BF, tag="selb_sb")
        mk80_sb = st.tile([128, 80], BF, tag="mk80_sb")
        v8 = [st.tile([8, F], BF, tag=f"v8_{i}", name=f"v8_{i}") for i in range(NBB)]
        vx4 = [st.tile([128, F], BF, tag=f"vx{i}", name=f"vx{i}") for i in range(NBB)]
        vall = st.tile([B_LOC, F], BF, tag="vall")
        vb8 = st.tile([8, NBB, F], BF, tag="vb8")
        og = st.tile([8, NBB, F], F32, tag="og")
        sv_sb = st.tile([8, NBB, F], F32, tag="sv_sb")

        # --- input loads; pipe order xt, ws0, bd0, ws1-3 so both the
        # iter-0 GEMM chain and build-e0 start as early as possible ---
        nc.sync.dma_start(out=xt_sb[:], in_=xt_d[:])
        nc.scalar.dma_start(out=ws_sb[:, 0:18, :], in_=ws_d[:, 0:18, :])
        bdt0 = bdp.tile([128, 9, NBB, 128], BD_DT, tag="bdt")
        nc.sync.dma_start(out=bdt0[:], in_=bd_d[0])
        for wc in range(1, 4):
            nc.scalar.dma_start(
                out=ws_sb[:, wc * 18:(wc + 1) * 18, :],
                in_=ws_d[:, wc * 18:(wc + 1) * 18, :])
        nc.sync.dma_start(out=selb_sb[:], in_=selb_d[:])
        nc.sync.dma_start(out=sel_sb[:], in_=sel_d[:])
        nc.scalar.dma_start(out=msk_sb[:], in_=msk_d[:])
        nc.scalar.dma_start(out=o80_sb[:], in_=o80_d[:])
        nc.sync.dma_start(out=mk80_sb[:], in_=mk80_d[:])

        # --- iter-0 stage 1: s0 = 0.1*sum_i u_hat as one GEMM chain over the
        # full (i,k) contraction: xt[128,(iblk),32] x ws[128,(iblk),160] ---
        ps0 = ps0p.tile([B_LOC, F], F32, tag="ps0", name="ps0")
        for j in range(NIB):
            nc.tensor.matmul(
                ps0[:], lhsT=xt_sb[:, j, :], rhs=ws_sb[:, j, :],
                start=(j == 0), stop=(j == NIB - 1))

        def load_vx(bblk, first):
            pvx = pvxp.tile([128, F], F32, tag="pvx")
            if first:
                nc.tensor.matmul(
                    pvx[:], lhsT=selb_sb[:, bblk, :], rhs=vall[:],
                    start=True, stop=True)
            else:
                nc.tensor.matmul(
                    pvx[:], lhsT=sel_sb[:], rhs=v8[bblk][:],
                    start=True, stop=True)
            nc.scalar.copy(vx4[bblk][:], pvx[:])

        def agreement_slice(bblk, j0, j1, first, eng):
            """blg[:, bblk, j0:j1, :] (+)= sum_d u_hat * v for a j-slice."""
            w = j1 - j0
            pool = eng is nc.gpsimd
            y2 = y2p.tile([128, w, F], BF, tag="y2p3" if pool else "y2")
            eng.tensor_mul(
                y2[:], u_hat[:, j0:j1, bblk, :],
                vx4[bblk][:].unsqueeze(1).broadcast_to((128, w, F)))
            # in-place pairwise-add tree over d (fp16, 2x mode)
            y2v = y2[:].rearrange("p j (o d) -> p j o d", d=OD)
            eng.tensor_add(
                y2v[:, :, :, 0:8], y2v[:, :, :, 0:8], y2v[:, :, :, 8:16])
            eng.tensor_add(
                y2v[:, :, :, 0:4], y2v[:, :, :, 0:4], y2v[:, :, :, 4:8])
            eng.tensor_add(
                y2v[:, :, :, 0:2], y2v[:, :, :, 0:2], y2v[:, :, :, 2:4])
            dst = blg[:, bblk, j0:j1, :]
            if first:
                eng.tensor_add(dst, y2v[:, :, :, 0], y2v[:, :, :, 1])
            else:
                ts = tsp.tile([128, w, OC], BF, tag="tsp3" if pool else "ts")
                eng.tensor_add(ts[:], y2v[:, :, :, 0], y2v[:, :, :, 1])
                eng.tensor_add(dst, dst, ts[:])


        # --- u_hat build: one matmul per (iblk, bblk), K=128=(ii,k), N=160;
        # PSUM drained to SBUF across ACT/DVE/Pool.  Iter-0 squash runs first
        # so each e-chunk's agreement slices interleave with the build. ---
        _squash(nc, smp, ps0[:], 0.1, vall)
        for bblk in range(NBB):
            load_vx(bblk, first=True)
        drain_seq = [nc.scalar, nc.scalar, nc.scalar, nc.scalar, nc.scalar,
                     nc.scalar, nc.scalar, nc.scalar, nc.scalar, nc.vector]
        ndrain = 0
        nagr = 0
        for e in range(8):
            if e == 0:
                bdt = bdt0
            else:
                bdt = bdp.tile([128, 9, NBB, 128], BD_DT, tag="bdt")
                nc.gpsimd.dma_start(out=bdt[:], in_=bd_d[e])
            for j in range(9):
                iblk = e * 9 + j
                for h in range(2):
                    ps = pbig.tile([128, 2, F], F32, tag="pbig")
                    for bb in range(2):
                        nc.tensor.matmul(
                            ps[:, bb, :], lhsT=bdt[:, j, h * 2 + bb, :],
                            rhs=ws_sb[:, iblk, :], start=True, stop=True)
                    if ndrain < 36:
                        # before iter-0 agreement arrives DVE is idle:
                        # split the early drains evenly
                        eng = (nc.scalar, nc.vector)[ndrain % 2]
                    else:
                        eng = drain_seq[ndrain % len(drain_seq)]
                    ndrain += 1
                    if eng is nc.scalar:
                        eng.copy(u_hat[:, iblk, h * 2:(h + 1) * 2, :], ps[:])
                    else:
                        eng.tensor_copy(u_hat[:, iblk, h * 2:(h + 1) * 2, :], ps[:])
            # iter-0 agreement in 18-j sets (every 2nd e-chunk): halves the
            # per-op fixed cost on DVE vs 9-j sets
            if e % 2 == 1:
                for bblk in range(NBB):
                    eng = nc.gpsimd if bblk == (e // 2) % 4 else nc.vector
                    nagr += 1
                    agreement_slice(bblk, (e - 1) * 9, (e + 1) * 9, True, eng)

        # --- iters 1, 2: software-pipelined across bblks and iterations.
        # softmax+cbt feed the PE stage-1 stream; each bblk's iter-2 chain
        # starts as soon as its iter-1 agreement lands. ---
        def softmax_cbt(it, bblk):
            nc.scalar.activation(
                c_sb[:, bblk, :, :], blg[:, bblk, :, :], AF.Exp)
            sm = smp.tile([128, NIB], F32, tag="sm")
            nc.vector.tensor_reduce(
                sm[:], c_sb[:, bblk, :, :], axis=AX.X, op=mybir.AluOpType.add)
            rr = smp.tile([128, NIB], BF, tag="rr")
            with nc.allow_low_precision(reason="softmax denom recip in fp16"):
                nc.vector.reciprocal(rr[:], sm[:])
            cn = cnp.tile([128, NIB, OC], BF, tag="cn")
            nc.vector.tensor_mul(
                cn[:], c_sb[:, bblk, :, :],
                rr[:].unsqueeze(-1).broadcast_to((128, NIB, OC)))
            eng = nc.vector
            cbt = cbp.tile([128, NIB, 80], BF, tag=f"cbt{bblk % 2}")
            eng.tensor_mul(
                cbt[:],
                cn[:].unsqueeze(2).broadcast_to((128, NIB, 8, OC)),
                mk80_sb[:].rearrange("p (b o) -> p b o", o=OC).unsqueeze(1)
                .broadcast_to((128, NIB, 8, OC)))
            ps1 = ps1p.tile([80, F], F32, tag="ps1", name=f"ps1_{it}_{bblk}")
            for j in range(NIB):
                nc.tensor.matmul(
                    ps1[:], lhsT=cbt[:, j, :],
                    rhs=u_hat[:, j, bblk, :],
                    start=(j == 0), stop=(j == NIB - 1))
            return ps1

        def extract_s(ps1):
            mskd = mkp.tile([80, F], BF, tag="mskd")
            nc.vector.tensor_mul(mskd[:], ps1[:], msk_sb[:])
            psv = psvp.tile([8, F], F32, tag="psv")
            nc.tensor.matmul(
                psv[:], lhsT=o80_sb[:], rhs=mskd[:], start=True, stop=True)
            return psv

        def batched_squash(dst):
            """dst[8, NBB, F] = squash(sv_sb) over all 4 b-blocks at once:
            8 ops instead of 32, cutting the latency-bound serial chain."""
            sq4 = st.tile([8, NBB, F], F32, tag="sq4")
            nc.vector.tensor_mul(sq4[:], sv_sb[:], sv_sb[:])
            n24 = st.tile([8, NBB, OC], F32, tag="n24")
            nc.vector.tensor_reduce(
                n24[:], sq4[:].rearrange("p b (o d) -> p b o d", d=OD),
                axis=AX.X, op=mybir.AluOpType.add)
            n14 = st.tile([8, NBB, OC], F32, tag="n14")
            nc.vector.tensor_scalar_add(n14[:], n24[:], 1.0)
            sn4 = st.tile([8, NBB, OC], F32, tag="sn4")
            nc.scalar.sqrt(sn4[:], n24[:])
            t14 = st.tile([8, NBB, OC], F32, tag="t14")
            nc.vector.tensor_mul(t14[:], n14[:], sn4[:])
            r14 = st.tile([8, NBB, OC], F32, tag="r14")
            nc.vector.reciprocal(r14[:], t14[:])
            f14 = st.tile([8, NBB, OC], F32, tag="f14")
            nc.vector.tensor_mul(f14[:], n24[:], r14[:])
            nc.vector.tensor_mul(
                dst[:].rearrange("p b (o d) -> p b o d", d=OD),
                sv_sb[:].rearrange("p b (o d) -> p b o d", d=OD),
                f14[:].unsqueeze(-1).broadcast_to((8, NBB, OC, OD)))

        for it in (1, 2):
            ps1s = [softmax_cbt(it, bb) for bb in range(NBB)]
            for bblk in range(NBB):
                psv = extract_s(ps1s[bblk])
                if it == 1:
                    _squash(nc, smp, psv[:], 1.0, v8[bblk])
                    load_vx(bblk, first=False)
                    agreement_slice(bblk, 0, 27, False, nc.vector)
                    agreement_slice(bblk, 27, 54, False, nc.vector)
                    agreement_slice(bblk, 54, 72, False, nc.gpsimd)
                else:
                    nc.scalar.copy(sv_sb[:, bblk, :], psv[:])
            if it == 2:
                batched_squash(og)
        nc.gpsimd.dma_start(
            out=out_d[:].rearrange("(bb bp) f -> bp bb f", bp=8), in_=og[:])

    if split_waits:
        _split_multiwait(nc)
    return nc


def _host_inputs(x, W):
    """Per-core input maps from full x [256,1152,8] f32, W [1,1152,10,16,8] f32."""
    bf = np.float16
    f8 = mybir.dt.np(BD_DT)
    W0 = np.asarray(W[0], dtype=np.float32)
    # ws[q=(ii,k), j, (o,d)] = W[j*16+ii, o, d, k]
    ws = np.ascontiguousarray(
        W0.reshape(NIB, 16, OC, OD, ID).transpose(1, 4, 0, 2, 3)
        .reshape(128, NIB, F)).astype(bf)
    msk = np.zeros((80, F), dtype=bf)
    for bpp in range(8):
        for o in range(OC):
            msk[bpp * 10 + o, o * OD:(o + 1) * OD] = 1.0
    o80 = np.zeros((80, 8), dtype=bf)
    for p in range(80):
        o80[p, p // 10] = 1.0
    sel = np.zeros((8, 128), dtype=bf)
    for p in range(128):
        sel[p // 16, p] = 1.0
    mk80 = np.zeros((128, 80), dtype=bf)
    for p in range(128):
        mk80[p, (p // 16) * 10:(p // 16) * 10 + 10] = 1.0
    selb = np.zeros((B_LOC, NBB, 128), dtype=bf)
    for bb in range(NBB):
        for p in range(128):
            selb[bb * 8 + p // 16, bb, p] = 1.0

    in_maps = []
    for c in range(N_CORES):
        xc = np.asarray(x[c * B_LOC:(c + 1) * B_LOC], dtype=np.float32)
        # bd[e, q=(ii,k), j, bb, m=(bp,ii')] = x[bb*8+bp, (e*9+j)*16+ii, k] iff ii'==ii
        r = xc.reshape(NBB, 8, 8, 9, 16, ID)          # [bb, bp, e, j, ii, k]
        bd6 = np.zeros((8, 16, ID, 9, NBB, 8, 16), dtype=np.float32)
        for ii in range(16):
            bd6[:, ii, :, :, :, :, ii] = r[:, :, :, :, ii, :].transpose(2, 4, 3, 0, 1)
        bd = np.ascontiguousarray(bd6.reshape(8, 128, 9, NBB, 128)).astype(f8)
        # xt[q=(ii,k), iblk, b] = x[b, iblk*16+ii, k]
        xt = np.ascontiguousarray(
            xc.reshape(B_LOC, NIB, 16, ID).transpose(2, 3, 1, 0)
            .reshape(128, NIB, B_LOC)).astype(bf)
        in_maps.append(
            {"bd": bd, "xt": xt, "ws": ws, "msk": msk, "o80": o80, "sel": sel,
             "mk80": mk80, "selb": selb})
    return in_maps


_NC_CACHE = {}


def kernel(x, W):
    from concourse.bass_utils import run_bass_kernel_spmd

    if "nc" not in _NC_CACHE:
        _NC_CACHE["nc"] = build_program()
    nc = _NC_CACHE["nc"]
    in_maps = _host_inputs(x, W)
    res = run_bass_kernel_spmd(nc, in_maps, core_ids=list(range(N_CORES)))
    out = np.concatenate([r["out"] for r in res.results], axis=0)
    return out.reshape(B_FULL, OC, OD).astype(np.float32)


if __name__ == "__main__":
    nc = build_program()
    print("program built ok")


# revision 31
# speedup vs baseline: 1.1211x; 1.0020x over previous
"""Trainium2 Bass kernel for a CapsuleNet dynamic-routing layer (v2).

Math (per batch element b):
    u_hat[b,i,o,d] = sum_k W[i,o,d,k] * x[b,i,k]      # B=256, IC=1152, OC=10, OD=16, ID=8
    b_log = 0
    for it in 0..2:
        c = softmax(b_log, axis=o)
        s[b,o,d] = sum_i c[b,i,o] * u_hat[b,i,o,d]
        v = squash(s)
        if it < 2: b_log += sum_d u_hat * v

Sharding: data-parallel over B across 8 cores (32 local rows), W replicated.

v2 changes vs v1 (cost-model driven; ~272us -> ~205us modeled):
  - W loaded once (chunked HWDGE DMAs) instead of 16 Pool-queue DMAs;
    iter-0 s = 0.1*sum_i u_hat is ONE accumulating 72-matmul GEMM (M=32)
    with a single batched squash, instead of 288 M=8 matmuls.
  - c block-diag built by a masked broadcast-multiply on DVE (2x mode)
    instead of 64 Pool-queue scatter DMAs; the mask writes the zeros so
    no memsets or persistent diag buffers are needed.
  - blg/c stored o-last so agreement writes and softmax run packed (2x).
  - PSUM drains phase-dependent: 1:1 ACT:DVE for the first 36 (DVE is
    idle before iter-0 agreement arrives), then 9:1 (GPSIMD cannot touch
    PSUM on HW; an op may read at most one PSUM operand); agreement
    split DVE/Pool ~75/25.
  - iter-0 agreement interleaved with the u_hat build per 2 e-chunks;
    iters 1-2 run softmax+cbt for all b-blocks first so the PE stage-1
    stream overlaps the extraction/agreement wave; the final 4 squashes
    are batched into one 8-op chain; one merged SWDGE output DMA.

  NOTE: fp8 for x/W was tried and is mathematically unsound here: in
  s = sum_i c*u both signal and quantization error grow as sqrt(N), so
  the ~3% per-element fp8 error lands on the output unattenuated.
"""

import sys

sys.path.insert(0, "/opt/trn_rl_repo")

from contextlib import ExitStack

import numpy as np

import concourse.bass as bass
import concourse.tile as tile
from concourse import mybir

BF = mybir.dt.float16
F8 = mybir.dt.float8e4
F32 = mybir.dt.float32
import os
BD_DT = F8 if os.environ.get("BD_FP8", "0") == "1" else BF
AX = mybir.AxisListType
AF = mybir.ActivationFunctionType

N_CORES = 8
B_FULL, IC, OC, OD, ID = 256, 1152, 10, 16, 8
B_LOC = B_FULL // N_CORES          # 32
NIB = IC // 16                     # 72 i-blocks of 16
NBB = B_LOC // 8                   # 4 b-blocks of 8
F = OC * OD                        # 160

def _squash(nc, smp, ps, scale, vout):
    """vout = squash(scale * ps) with ps an [P, 160] psum slab (f32).

    squash(s) = (n^2/(1+n^2)) * s/(n + 1e-8),  n = ||s||_2 over d.
    The 1e-8 is dropped (n >= 1e-3 in routing; rel err < 1e-5).
    """
    P = ps.shape[0]
    sq = smp.tile([P, F], F32, tag="sq")
    nc.scalar.activation(sq[:], ps[:], AF.Square, scale=float(scale))
    n2 = smp.tile([P, OC], F32, tag="n2")
    nc.vector.tensor_reduce(
        n2[:], sq[:].rearrange("p (o d) -> p o d", d=OD), axis=AX.X,
        op=mybir.AluOpType.add)
    n1 = smp.tile([P, OC], F32, tag="n1")
    nc.vector.tensor_scalar_add(n1[:], n2[:], 1.0)
    sn = smp.tile([P, OC], F32, tag="sn")
    nc.scalar.sqrt(sn[:], n2[:])
    t1 = smp.tile([P, OC], F32, tag="t1")
    nc.vector.tensor_mul(t1[:], n1[:], sn[:])
    r1 = smp.tile([P, OC], F32, tag="r1")
    nc.vector.reciprocal(r1[:], t1[:])
    f1 = smp.tile([P, OC], F32, tag="f1")
    nc.vector.tensor_mul(f1[:], n2[:], r1[:])
    if scale != 1.0:
        nc.vector.tensor_scalar_mul(f1[:], f1[:], float(scale))
    nc.vector.tensor_mul(
        vout[:].rearrange("p (o d) -> p o d", d=OD),
        ps[:].rearrange("p (o d) -> p o d", d=OD),
        f1[:].unsqueeze(-1).broadcast_to((P, OC, OD)))


def _split_multiwait(nc):
    """Walrus encodes at most ONE semaphore wait per engine/DMA instruction.
    Hoist excess waits onto same-engine NoOps placed directly before the
    instruction.  HWDGE DMAs can't be gated that way - assert instead."""
    for fn in nc.m.functions:
        for bb in fn.blocks:
            out = []
            k = 0
            for ins in bb.instructions:
                si = ins.sync_info
                waits = list(si.on_wait) if si is not None and si.on_wait else []
                limit = 1
                if ins.opcode == "DMACopy":
                    q = str(getattr(ins, "queue", "") or "")
                    if "HW" in q and len(waits) > 1:
                        raise AssertionError(
                            f"HWDGE DMA {ins.name} has {len(waits)} waits: {ins}")
                if len(waits) > limit:
                    for w in waits[:-limit]:
                        nop = mybir.InstNoOp(name=f"{ins.name}-wn{k}", ins=[], outs=[])
                        k += 1
                        nop.engine = ins.engine
                        nop.sync_info = mybir.SyncInfo(on_wait=[w], on_update=[])
                        out.append(nop)
                    ins.sync_info = mybir.SyncInfo(
                        on_wait=waits[-limit:],
                        on_update=list(si.on_update) if si.on_update else [])
                out.append(ins)
            bb.instructions = out


def build_program(split_waits=True):
    nc = bass.Bass()
    bd_d = nc.declare_dram_parameter("bd", [8, 128, 9, NBB, 128], BD_DT, isOutput=False)
    xt_d = nc.declare_dram_parameter("xt", [128, NIB, B_LOC], BF, isOutput=False)
    ws_d = nc.declare_dram_parameter("ws", [128, NIB, F], BF, isOutput=False)
    msk_d = nc.declare_dram_parameter("msk", [80, F], BF, isOutput=False)
    o80_d = nc.declare_dram_parameter("o80", [80, 8], BF, isOutput=False)
    sel_d = nc.declare_dram_parameter("sel", [8, 128], BF, isOutput=False)
    selb_d = nc.declare_dram_parameter("selb", [B_LOC, NBB, 128], BF, isOutput=False)
    mk80_d = nc.declare_dram_parameter("mk80", [128, 80], BF, isOutput=False)
    out_d = nc.declare_dram_parameter("out", [B_LOC, F], F32, isOutput=True)

    with ExitStack() as ctx:
        tc = ctx.enter_context(tile.TileContext(nc))
        st = ctx.enter_context(tc.tile_pool(name="st", bufs=1))
        bdp = ctx.enter_context(tc.tile_pool(name="bdp", bufs=2))
        y2p = ctx.enter_context(tc.tile_pool(name="y2p", bufs=1))
        tsp = ctx.enter_context(tc.tile_pool(name="tsp", bufs=1))
        mkp = ctx.enter_context(tc.tile_pool(name="mkp", bufs=2))
        cnp = ctx.enter_context(tc.tile_pool(name="cnp", bufs=1))
        cbp = ctx.enter_context(tc.tile_pool(name="cbp", bufs=1))
        vxp = ctx.enter_context(tc.tile_pool(name="vxp", bufs=2))
        smp = ctx.enter_context(tc.tile_pool(name="smp", bufs=4))
        pbig = ctx.enter_context(tc.tile_pool(name="pbig", bufs=3, space="PSUM"))
        ps1p = ctx.enter_context(tc.tile_pool(name="ps1p", bufs=2, space="PSUM"))
        ps0p = ctx.enter_context(tc.tile_pool(name="ps0p", bufs=1, space="PSUM"))
        pvxp = ctx.enter_context(tc.tile_pool(name="pvxp", bufs=1, space="PSUM"))
        psvp = ctx.enter_context(tc.tile_pool(name="psvp", bufs=1, space="PSUM"))

        # --- persistent tiles ---
        u_hat = st.tile([128, NIB, NBB, F], BF, tag="u_hat")
        ws_sb = st.tile([128, NIB, F], BF, tag="ws_sb")
        xt_sb = st.tile([128, NIB, B_LOC], BF, tag="xt_sb")
        blg = st.tile([128, NBB, NIB, OC], BF, tag="blg")
        c_sb = st.tile([128, NBB, NIB, OC], BF, tag="c_sb")
        msk_sb = st.tile([80, F], BF, tag="msk_sb")
        o80_sb = st.tile([80, 8], BF, tag="o80_sb")
        sel_sb = st.tile([8, 128], BF, tag="sel_sb")
        selb_sb = st.tile([B_LOC, NBB, 128], BF, tag="selb_sb")
        mk80_sb = st.tile([128, 80], BF, tag="mk80_sb")
        v8 = [st.tile([8, F], BF, tag=f"v8_{i}", name=f"v8_{i}") for i in range(NBB)]
        vx4 = [st.tile([128, F], BF, tag=f"vx{i}", name=f"vx{i}") for i in range(NBB)]
        vall = st.tile([B_LOC, F], BF, tag="vall")
        vb8 = st.tile([8, NBB, F], BF, tag="vb8")
        og = st.tile([8, NBB, F], F32, tag="og")
        sv_sb = st.tile([8, NBB, F], F32, tag="sv_sb")

        # --- input loads; pipe order xt, ws0, bd0, ws1-3 so both the
        # iter-0 GEMM chain and build-e0 start as early as possible ---
        nc.sync.dma_start(out=xt_sb[:], in_=xt_d[:])
        nc.scalar.dma_start(out=ws_sb[:, 0:18, :], in_=ws_d[:, 0:18, :])
        bdt0 = bdp.tile([128, 9, NBB, 128], BD_DT, tag="bdt")
        nc.sync.dma_start(out=bdt0[:], in_=bd_d[0])
        for wc in range(1, 4):
            nc.scalar.dma_start(
                out=ws_sb[:, wc * 18:(wc + 1) * 18, :],
                in_=ws_d[:, wc * 18:(wc + 1) * 18, :])
        nc.sync.dma_start(out=selb_sb[:], in_=selb_d[:])
        nc.sync.dma_start(out=sel_sb[:], in_=sel_d[:])
        nc.scalar.dma_start(out=msk_sb[:], in_=msk_d[:])
        nc.scalar.dma_start(out=o80_sb[:], in_=o80_d[:])
        nc.sync.dma_start(out=mk80_sb[:], in_=mk80_d[:])

        # --- iter-0 stage 1: s0 = 0.1*sum_i u_hat as one GEMM chain over the
        # full (i,k) contraction.  Only the first ws chunk's 18 matmuls are
        # emitted here; the rest interleave after build-e0 (the squash is
        # arrival-bound on the last ws chunk either way, and this starts the
        # build drains ~6us earlier). ---
        ps0 = ps0p.tile([B_LOC, F], F32, tag="ps0", name="ps0")
        for j in range(18):
            nc.tensor.matmul(
                ps0[:], lhsT=xt_sb[:, j, :], rhs=ws_sb[:, j, :],
                start=(j == 0), stop=False, skip_group_check=True)

        def load_vx(bblk, first):
            pvx = pvxp.tile([128, F], F32, tag="pvx")
            if first:
                nc.tensor.matmul(
                    pvx[:], lhsT=selb_sb[:, bblk, :], rhs=vall[:],
                    start=True, stop=True)
            else:
                nc.tensor.matmul(
                    pvx[:], lhsT=sel_sb[:], rhs=v8[bblk][:],
                    start=True, stop=True)
            nc.scalar.copy(vx4[bblk][:], pvx[:])

        def agreement_slice(bblk, j0, j1, first, eng):
            """blg[:, bblk, j0:j1, :] (+)= sum_d u_hat * v for a j-slice."""
            w = j1 - j0
            pool = eng is nc.gpsimd
            y2 = y2p.tile([128, w, F], BF, tag="y2p3" if pool else "y2")
            eng.tensor_mul(
                y2[:], u_hat[:, j0:j1, bblk, :],
                vx4[bblk][:].unsqueeze(1).broadcast_to((128, w, F)))
            # in-place pairwise-add tree over d (fp16, 2x mode)
            y2v = y2[:].rearrange("p j (o d) -> p j o d", d=OD)
            eng.tensor_add(
                y2v[:, :, :, 0:8], y2v[:, :, :, 0:8], y2v[:, :, :, 8:16])
            eng.tensor_add(
                y2v[:, :, :, 0:4], y2v[:, :, :, 0:4], y2v[:, :, :, 4:8])
            eng.tensor_add(
                y2v[:, :, :, 0:2], y2v[:, :, :, 0:2], y2v[:, :, :, 2:4])
            dst = blg[:, bblk, j0:j1, :]
            if first:
                eng.tensor_add(dst, y2v[:, :, :, 0], y2v[:, :, :, 1])
            else:
                ts = tsp.tile([128, w, OC], BF, tag="tsp3" if pool else "ts")
                eng.tensor_add(ts[:], y2v[:, :, :, 0], y2v[:, :, :, 1])
                eng.tensor_add(dst, dst, ts[:])


        # --- u_hat build: one matmul per (iblk, bblk), K=128=(ii,k), N=160;
        # PSUM drained to SBUF across ACT/DVE/Pool.  Iter-0 squash runs first
        # so each e-chunk's agreement slices interleave with the build. ---
        drain_seq = [nc.scalar, nc.scalar, nc.scalar, nc.scalar, nc.scalar,
                     nc.scalar, nc.scalar, nc.scalar, nc.scalar, nc.vector]
        ndrain = 0
        nagr = 0
        for e in range(8):
            if e == 0:
                bdt = bdt0
            else:
                bdt = bdp.tile([128, 9, NBB, 128], BD_DT, tag="bdt")
                nc.gpsimd.dma_start(out=bdt[:], in_=bd_d[e])
            for j in range(9):
                iblk = e * 9 + j
                for h in range(2):
                    ps = pbig.tile([128, 2, F], F32, tag="pbig")
                    for bb in range(2):
                        nc.tensor.matmul(
                            ps[:, bb, :], lhsT=bdt[:, j, h * 2 + bb, :],
                            rhs=ws_sb[:, iblk, :], start=True, stop=True)
                    if ndrain < 36:
                        # before iter-0 agreement arrives DVE is idle:
                        # split the early drains evenly
                        eng = (nc.scalar, nc.vector)[ndrain % 2]
                    else:
                        eng = drain_seq[ndrain % len(drain_seq)]
                    ndrain += 1
                    if eng is nc.scalar:
                        eng.copy(u_hat[:, iblk, h * 2:(h + 1) * 2, :], ps[:])
                    else:
                        eng.tensor_copy(u_hat[:, iblk, h * 2:(h + 1) * 2, :], ps[:])
            if e == 0:
                # rest of the iter-0 GEMM, then its squash + vx broadcasts
                for j in range(18, NIB):
                    nc.tensor.matmul(
                        ps0[:], lhsT=xt_sb[:, j, :], rhs=ws_sb[:, j, :],
                        start=False, stop=(j == NIB - 1), skip_group_check=True)
                _squash(nc, smp, ps0[:], 0.1, vall)
                for bblk in range(NBB):
                    load_vx(bblk, first=True)
            # iter-0 agreement in 18-j sets (every 2nd e-chunk): halves the
            # per-op fixed cost on DVE vs 9-j sets
            if e % 2 == 1:
                for bblk in range(NBB):
                    eng = nc.gpsimd if bblk == (e // 2) % 4 else nc.vector
                    nagr += 1
                    agreement_slice(bblk, (e - 1) * 9, (e + 1) * 9, True, eng)

        # --- iters 1, 2: software-pipelined across bblks and iterations.
        # softmax+cbt feed the PE stage-1 stream; each bblk's iter-2 chain
        # starts as soon as its iter-1 agreement lands. ---
        def softmax_cbt(it, bblk):
            nc.scalar.activation(
                c_sb[:, bblk, :, :], blg[:, bblk, :, :], AF.Exp)
            sm = smp.tile([128, NIB], F32, tag="sm")
            nc.vector.tensor_reduce(
                sm[:], c_sb[:, bblk, :, :], axis=AX.X, op=mybir.AluOpType.add)
            rr = smp.tile([128, NIB], BF, tag="rr")
            with nc.allow_low_precision(reason="softmax denom recip in fp16"):
                nc.vector.reciprocal(rr[:], sm[:])
            cn = cnp.tile([128, NIB, OC], BF, tag="cn")
            nc.vector.tensor_mul(
                cn[:], c_sb[:, bblk, :, :],
                rr[:].unsqueeze(-1).broadcast_to((128, NIB, OC)))
            eng = nc.vector
            cbt = cbp.tile([128, NIB, 80], BF, tag=f"cbt{bblk % 2}")
            eng.tensor_mul(
                cbt[:],
                cn[:].unsqueeze(2).broadcast_to((128, NIB, 8, OC)),
                mk80_sb[:].rearrange("p (b o) -> p b o", o=OC).unsqueeze(1)
                .broadcast_to((128, NIB, 8, OC)))
            ps1 = ps1p.tile([80, F], F32, tag="ps1", name=f"ps1_{it}_{bblk}")
            for j in range(NIB):
                nc.tensor.matmul(
                    ps1[:], lhsT=cbt[:, j, :],
                    rhs=u_hat[:, j, bblk, :],
                    start=(j == 0), stop=(j == NIB - 1))
            return ps1

        def extract_s(ps1):
            mskd = mkp.tile([80, F], BF, tag="mskd")
            nc.vector.tensor_mul(mskd[:], ps1[:], msk_sb[:])
            psv = psvp.tile([8, F], F32, tag="psv")
            nc.tensor.matmul(
                psv[:], lhsT=o80_sb[:], rhs=mskd[:], start=True, stop=True)
            return psv

        def batched_squash(dst):
            """dst[8, NBB, F] = squash(sv_sb) over all 4 b-blocks at once:
            8 ops instead of 32, cutting the latency-bound serial chain."""
            sq4 = st.tile([8, NBB, F], F32, tag="sq4")
            nc.vector.tensor_mul(sq4[:], sv_sb[:], sv_sb[:])
            n24 = st.tile([8, NBB, OC], F32, tag="n24")
            nc.vector.tensor_reduce(
                n24[:], sq4[:].rearrange("p b (o d) -> p b o d", d=OD),
                axis=AX.X, op=mybir.AluOpType.add)
            n14 = st.tile([8, NBB, OC], F32, tag="n14")
            nc.vector.tensor_scalar_add(n14[:], n24[:], 1.0)
            sn4 = st.tile([8, NBB, OC], F32, tag="sn4")
            nc.scalar.sqrt(sn4[:], n24[:])
            t14 = st.tile([8, NBB, OC], F32, tag="t14")
            nc.vector.tensor_mul(t14[:], n14[:], sn4[:])
            r14 = st.tile([8, NBB, OC], F32, tag="r14")
            nc.vector.reciprocal(r14[:], t14[:])
            f14 = st.tile([8, NBB, OC], F32, tag="f14")
            nc.vector.tensor_mul(f14[:], n24[:], r14[:])
            nc.vector.tensor_mul(
                dst[:].rearrange("p b (o d) -> p b o d", d=OD),
                sv_sb[:].rearrange("p b (o d) -> p b o d", d=OD),
                f14[:].unsqueeze(-1).broadcast_to((8, NBB, OC, OD)))

        for it in (1, 2):
            ps1s = [softmax_cbt(it, bb) for bb in range(NBB)]
            for bblk in range(NBB):
                psv = extract_s(ps1s[bblk])
                if it == 1:
                    _squash(nc, smp, psv[:], 1.0, v8[bblk])
                    load_vx(bblk, first=False)
                    agreement_slice(bblk, 0, 27, False, nc.vector)
                    agreement_slice(bblk, 27, 54, False, nc.vector)
                    agreement_slice(bblk, 54, 72, False, nc.gpsimd)
                else:
                    nc.scalar.copy(sv_sb[:, bblk, :], psv[:])
            if it == 2:
                batched_squash(og)
        nc.gpsimd.dma_start(
            out=out_d[:].rearrange("(bb bp) f -> bp bb f", bp=8), in_=og[:])

    if split_waits:
        _split_multiwait(nc)
    return nc


def _host_inputs(x, W):
    """Per-core input maps from full x [256,1152,8] f32, W [1,1152,10,16,8] f32."""
    bf = np.float16
    f8 = mybir.dt.np(BD_DT)
    W0 = np.asarray(W[0], dtype=np.float32)
    # ws[q=(ii,k), j, (o,d)] = W[j*16+ii, o, d, k]
    ws = np.ascontiguousarray(
        W0.reshape(NIB, 16, OC, OD, ID).transpose(1, 4, 0, 2, 3)
        .reshape(128, NIB, F)).astype(bf)
    msk = np.zeros((80, F), dtype=bf)
    for bpp in range(8):
        for o in range(OC):
            msk[bpp * 10 + o, o * OD:(o + 1) * OD] = 1.0
    o80 = np.zeros((80, 8), dtype=bf)
    for p in range(80):
        o80[p, p // 10] = 1.0
    sel = np.zeros((8, 128), dtype=bf)
    for p in range(128):
        sel[p // 16, p] = 1.0
    mk80 = np.zeros((128, 80), dtype=bf)
    for p in range(128):
        mk80[p, (p // 16) * 10:(p // 16) * 10 + 10] = 1.0
    selb = np.zeros((B_LOC, NBB, 128), dtype=bf)
    for bb in range(NBB):
        for p in range(128):
            selb[bb * 8 + p // 16, bb, p] = 1.0

    in_maps = []
    for c in range(N_CORES):
        xc = np.asarray(x[c * B_LOC:(c + 1) * B_LOC], dtype=np.float32)
        # bd[e, q=(ii,k), j, bb, m=(bp,ii')] = x[bb*8+bp, (e*9+j)*16+ii, k] iff ii'==ii
        r = xc.reshape(NBB, 8, 8, 9, 16, ID)          # [bb, bp, e, j, ii, k]
        bd6 = np.zeros((8, 16, ID, 9, NBB, 8, 16), dtype=np.float32)
        for ii in range(16):
            bd6[:, ii, :, :, :, :, ii] = r[:, :, :, :, ii, :].transpose(2, 4, 3, 0, 1)
        bd = np.ascontiguousarray(bd6.reshape(8, 128, 9, NBB, 128)).astype(f8)
        # xt[q=(ii,k), iblk, b] = x[b, iblk*16+ii, k]
        xt = np.ascontiguousarray(
            xc.reshape(B_LOC, NIB, 16, ID).transpose(2, 3, 1, 0)
            .reshape(128, NIB, B_LOC)).astype(bf)
        in_maps.append(
            {"bd": bd, "xt": xt, "ws": ws, "msk": msk, "o80": o80, "sel": sel,
             "mk80": mk80, "selb": selb})
    return in_maps


_NC_CACHE = {}


def kernel(x, W):
    from concourse.bass_utils import run_bass_kernel_spmd

    if "nc" not in _NC_CACHE:
        _NC_CACHE["nc"] = build_program()
    nc = _NC_CACHE["nc"]
    in_maps = _host_inputs(x, W)
    res = run_bass_kernel_spmd(nc, in_maps, core_ids=list(range(N_CORES)))
    out = np.concatenate([r["out"] for r in res.results], axis=0)
    return out.reshape(B_FULL, OC, OD).astype(np.float32)


if __name__ == "__main__":
    nc = build_program()
    print("program built ok")


# revision 36
# speedup vs baseline: 1.1253x; 1.0037x over previous
"""Trainium2 Bass kernel for a CapsuleNet dynamic-routing layer (v2).

Math (per batch element b):
    u_hat[b,i,o,d] = sum_k W[i,o,d,k] * x[b,i,k]      # B=256, IC=1152, OC=10, OD=16, ID=8
    b_log = 0
    for it in 0..2:
        c = softmax(b_log, axis=o)
        s[b,o,d] = sum_i c[b,i,o] * u_hat[b,i,o,d]
        v = squash(s)
        if it < 2: b_log += sum_d u_hat * v

Sharding: data-parallel over B across 8 cores (32 local rows), W replicated.

v2 changes vs v1 (cost-model driven; ~272us -> ~199us modeled):
  - W loaded once (chunked HWDGE DMAs) instead of 16 Pool-queue DMAs;
    iter-0 s = 0.1*sum_i u_hat is ONE accumulating 72-matmul GEMM (M=32)
    with a single batched squash, instead of 288 M=8 matmuls.
  - c block-diag built by a masked broadcast-multiply on DVE (2x mode)
    instead of 64 Pool-queue scatter DMAs; the mask writes the zeros so
    no memsets or persistent diag buffers are needed.
  - blg/c stored o-last so agreement writes and softmax run packed (2x).
  - PSUM drains phase-dependent: 1:1 ACT:DVE for the first 36 (DVE is
    idle before iter-0 agreement arrives), then 9:1 (GPSIMD cannot touch
    PSUM on HW; an op may read at most one PSUM operand); agreement
    split DVE/Pool ~75/25.
  - iter-0 agreement interleaved with the u_hat build per 2 e-chunks;
    iters 1-2 run softmax+cbt for all b-blocks first so the PE stage-1
    stream overlaps the extraction/agreement wave; the final 4 squashes
    are batched into one 8-op chain; one merged SWDGE output DMA.

  NOTE: fp8 for x/W was tried and is mathematically unsound here: in
  s = sum_i c*u both signal and quantization error grow as sqrt(N), so
  the ~3% per-element fp8 error lands on the output unattenuated.
"""

import sys

sys.path.insert(0, "/opt/trn_rl_repo")

from contextlib import ExitStack

import numpy as np

import concourse.bass as bass
import concourse.tile as tile
from concourse import mybir

BF = mybir.dt.float16
F8 = mybir.dt.float8e4
F32 = mybir.dt.float32
import os
BD_DT = F8 if os.environ.get("BD_FP8", "0") == "1" else BF
AX = mybir.AxisListType
AF = mybir.ActivationFunctionType

N_CORES = 8
B_FULL, IC, OC, OD, ID = 256, 1152, 10, 16, 8
B_LOC = B_FULL // N_CORES          # 32
NIB = IC // 16                     # 72 i-blocks of 16
NBB = B_LOC // 8                   # 4 b-blocks of 8
F = OC * OD                        # 160

def _squash(nc, smp, ps, scale, vout):
    """vout = squash(scale * ps) with ps an [P, 160] psum slab (f32).

    squash(s) = (n^2/(1+n^2)) * s/(n + 1e-8),  n = ||s||_2 over d.
    The 1e-8 is dropped (n >= 1e-3 in routing; rel err < 1e-5).
    """
    P = ps.shape[0]
    sq = smp.tile([P, F], F32, tag="sq")
    nc.scalar.activation(sq[:], ps[:], AF.Square, scale=float(scale))
    n2 = smp.tile([P, OC], F32, tag="n2")
    nc.vector.tensor_reduce(
        n2[:], sq[:].rearrange("p (o d) -> p o d", d=OD), axis=AX.X,
        op=mybir.AluOpType.add)
    n1 = smp.tile([P, OC], F32, tag="n1")
    nc.vector.tensor_scalar_add(n1[:], n2[:], 1.0)
    sn = smp.tile([P, OC], F32, tag="sn")
    nc.scalar.sqrt(sn[:], n2[:])
    t1 = smp.tile([P, OC], F32, tag="t1")
    nc.vector.tensor_mul(t1[:], n1[:], sn[:])
    r1 = smp.tile([P, OC], F32, tag="r1")
    nc.vector.reciprocal(r1[:], t1[:])
    f1 = smp.tile([P, OC], F32, tag="f1")
    nc.vector.tensor_mul(f1[:], n2[:], r1[:])
    if scale != 1.0:
        nc.vector.tensor_scalar_mul(f1[:], f1[:], float(scale))
    nc.vector.tensor_mul(
        vout[:].rearrange("p (o d) -> p o d", d=OD),
        ps[:].rearrange("p (o d) -> p o d", d=OD),
        f1[:].unsqueeze(-1).broadcast_to((P, OC, OD)))


def _split_multiwait(nc):
    """Walrus encodes at most ONE semaphore wait per engine/DMA instruction.
    Hoist excess waits onto same-engine NoOps placed directly before the
    instruction.  HWDGE DMAs can't be gated that way - assert instead."""
    for fn in nc.m.functions:
        for bb in fn.blocks:
            out = []
            k = 0
            for ins in bb.instructions:
                si = ins.sync_info
                waits = list(si.on_wait) if si is not None and si.on_wait else []
                limit = 1
                if ins.opcode == "DMACopy":
                    q = str(getattr(ins, "queue", "") or "")
                    if "HW" in q and len(waits) > 1:
                        raise AssertionError(
                            f"HWDGE DMA {ins.name} has {len(waits)} waits: {ins}")
                if len(waits) > limit:
                    for w in waits[:-limit]:
                        nop = mybir.InstNoOp(name=f"{ins.name}-wn{k}", ins=[], outs=[])
                        k += 1
                        nop.engine = ins.engine
                        nop.sync_info = mybir.SyncInfo(on_wait=[w], on_update=[])
                        out.append(nop)
                    ins.sync_info = mybir.SyncInfo(
                        on_wait=waits[-limit:],
                        on_update=list(si.on_update) if si.on_update else [])
                out.append(ins)
            bb.instructions = out


def build_program(split_waits=True):
    nc = bass.Bass()
    bd_d = nc.declare_dram_parameter("bd", [8, 128, 9, NBB, 128], BD_DT, isOutput=False)
    xt_d = nc.declare_dram_parameter("xt", [128, NIB, B_LOC], BF, isOutput=False)
    ws_d = nc.declare_dram_parameter("ws", [128, NIB, F], BF, isOutput=False)
    msk_d = nc.declare_dram_parameter("msk", [80, F], BF, isOutput=False)
    o80_d = nc.declare_dram_parameter("o80", [80, 8], BF, isOutput=False)
    sel_d = nc.declare_dram_parameter("sel", [8, 128], BF, isOutput=False)
    selb_d = nc.declare_dram_parameter("selb", [B_LOC, NBB, 128], BF, isOutput=False)
    mk80_d = nc.declare_dram_parameter("mk80", [128, 80], BF, isOutput=False)
    out_d = nc.declare_dram_parameter("out", [B_LOC, F], F32, isOutput=True)

    with ExitStack() as ctx:
        tc = ctx.enter_context(tile.TileContext(nc))
        st = ctx.enter_context(tc.tile_pool(name="st", bufs=1))
        bdp = ctx.enter_context(tc.tile_pool(name="bdp", bufs=2))
        y2p = ctx.enter_context(tc.tile_pool(name="y2p", bufs=1))
        tsp = ctx.enter_context(tc.tile_pool(name="tsp", bufs=1))
        mkp = ctx.enter_context(tc.tile_pool(name="mkp", bufs=2))
        cnp = ctx.enter_context(tc.tile_pool(name="cnp", bufs=1))
        cbp = ctx.enter_context(tc.tile_pool(name="cbp", bufs=1))
        vxp = ctx.enter_context(tc.tile_pool(name="vxp", bufs=2))
        smp = ctx.enter_context(tc.tile_pool(name="smp", bufs=4))
        pbig = ctx.enter_context(tc.tile_pool(name="pbig", bufs=3, space="PSUM"))
        ps1p = ctx.enter_context(tc.tile_pool(name="ps1p", bufs=2, space="PSUM"))
        ps0p = ctx.enter_context(tc.tile_pool(name="ps0p", bufs=1, space="PSUM"))
        pvxp = ctx.enter_context(tc.tile_pool(name="pvxp", bufs=1, space="PSUM"))
        psvp = ctx.enter_context(tc.tile_pool(name="psvp", bufs=1, space="PSUM"))

        # --- persistent tiles ---
        u_hat = st.tile([128, NIB, NBB, F], BF, tag="u_hat")
        ws_sb = st.tile([128, NIB, F], BF, tag="ws_sb")
        xt_sb = st.tile([128, NIB, B_LOC], BF, tag="xt_sb")
        blg = st.tile([128, NBB, NIB, OC], BF, tag="blg")
        c_sb = st.tile([128, NBB, NIB, OC], BF, tag="c_sb")
        msk_sb = st.tile([80, F], BF, tag="msk_sb")
        o80_sb = st.tile([80, 8], BF, tag="o80_sb")
        sel_sb = st.tile([8, 128], BF, tag="sel_sb")
        selb_sb = st.tile([B_LOC, NBB, 128], BF, tag="selb_sb")
        mk80_sb = st.tile([128, 80], BF, tag="mk80_sb")
        v8 = [st.tile([8, F], BF, tag=f"v8_{i}", name=f"v8_{i}") for i in range(NBB)]
        vx4 = [st.tile([128, F], BF, tag=f"vx{i}", name=f"vx{i}") for i in range(NBB)]
        vall = st.tile([B_LOC, F], BF, tag="vall")
        vb8 = st.tile([8, NBB, F], BF, tag="vb8")
        og = st.tile([8, NBB, F], F32, tag="og")
        sv_sb = st.tile([8, NBB, F], F32, tag="sv_sb")

        # --- input loads; pipe order xt, ws0, bd0, ws1-3 so both the
        # iter-0 GEMM chain and build-e0 start as early as possible ---
        bdt0 = bdp.tile([128, 9, NBB, 128], BD_DT, tag="bdt")
        nc.sync.dma_start(out=bdt0[:], in_=bd_d[0])
        nc.scalar.dma_start(out=ws_sb[:, 0:18, :], in_=ws_d[:, 0:18, :])
        nc.sync.dma_start(out=xt_sb[:], in_=xt_d[:])
        for wc in range(1, 4):
            nc.scalar.dma_start(
                out=ws_sb[:, wc * 18:(wc + 1) * 18, :],
                in_=ws_d[:, wc * 18:(wc + 1) * 18, :])
        nc.sync.dma_start(out=selb_sb[:], in_=selb_d[:])
        nc.sync.dma_start(out=sel_sb[:], in_=sel_d[:])
        nc.scalar.dma_start(out=msk_sb[:], in_=msk_d[:])
        nc.scalar.dma_start(out=o80_sb[:], in_=o80_d[:])
        nc.sync.dma_start(out=mk80_sb[:], in_=mk80_d[:])

        # --- iter-0 stage 1: s0 = 0.1*sum_i u_hat as one GEMM chain over the
        # full (i,k) contraction.  Only the first ws chunk's 18 matmuls are
        # emitted here; the rest interleave after build-e0 (the squash is
        # arrival-bound on the last ws chunk either way, and this starts the
        # build drains ~6us earlier). ---
        ps0 = ps0p.tile([B_LOC, F], F32, tag="ps0", name="ps0")
        for j in range(18):
            nc.tensor.matmul(
                ps0[:], lhsT=xt_sb[:, j, :], rhs=ws_sb[:, j, :],
                start=(j == 0), stop=False, skip_group_check=True)

        def load_vx(bblk, first):
            pvx = pvxp.tile([128, F], F32, tag="pvx")
            if first:
                nc.tensor.matmul(
                    pvx[:], lhsT=selb_sb[:, bblk, :], rhs=vall[:],
                    start=True, stop=True)
            else:
                nc.tensor.matmul(
                    pvx[:], lhsT=sel_sb[:], rhs=v8[bblk][:],
                    start=True, stop=True)
            nc.scalar.copy(vx4[bblk][:], pvx[:])

        def agreement_slice(bblk, j0, j1, first, eng):
            """blg[:, bblk, j0:j1, :] (+)= sum_d u_hat * v for a j-slice."""
            w = j1 - j0
            pool = eng is nc.gpsimd
            y2 = y2p.tile([128, w, F], BF, tag="y2p3" if pool else "y2")
            eng.tensor_mul(
                y2[:], u_hat[:, j0:j1, bblk, :],
                vx4[bblk][:].unsqueeze(1).broadcast_to((128, w, F)))
            # in-place pairwise-add tree over d (fp16, 2x mode)
            y2v = y2[:].rearrange("p j (o d) -> p j o d", d=OD)
            eng.tensor_add(
                y2v[:, :, :, 0:8], y2v[:, :, :, 0:8], y2v[:, :, :, 8:16])
            eng.tensor_add(
                y2v[:, :, :, 0:4], y2v[:, :, :, 0:4], y2v[:, :, :, 4:8])
            eng.tensor_add(
                y2v[:, :, :, 0:2], y2v[:, :, :, 0:2], y2v[:, :, :, 2:4])
            dst = blg[:, bblk, j0:j1, :]
            if first:
                eng.tensor_add(dst, y2v[:, :, :, 0], y2v[:, :, :, 1])
            else:
                ts = tsp.tile([128, w, OC], BF, tag="tsp3" if pool else "ts")
                eng.tensor_add(ts[:], y2v[:, :, :, 0], y2v[:, :, :, 1])
                eng.tensor_add(dst, dst, ts[:])


        # --- u_hat build: one matmul per (iblk, bblk), K=128=(ii,k), N=160;
        # PSUM drained to SBUF across ACT/DVE/Pool.  Iter-0 squash runs first
        # so each e-chunk's agreement slices interleave with the build. ---
        drain_seq = [nc.scalar, nc.scalar, nc.scalar, nc.scalar, nc.scalar,
                     nc.scalar, nc.scalar, nc.scalar, nc.scalar, nc.vector]
        ndrain = 0
        nagr = 0
        for e in range(8):
            if e == 0:
                bdt = bdt0
            else:
                bdt = bdp.tile([128, 9, NBB, 128], BD_DT, tag="bdt")
                nc.gpsimd.dma_start(out=bdt[:], in_=bd_d[e])
            for j in range(9):
                iblk = e * 9 + j
                for h in range(2):
                    ps = pbig.tile([128, 2, F], F32, tag="pbig")
                    for bb in range(2):
                        nc.tensor.matmul(
                            ps[:, bb, :], lhsT=bdt[:, j, h * 2 + bb, :],
                            rhs=ws_sb[:, iblk, :], start=True, stop=True)
                    if ndrain < 36:
                        # before iter-0 agreement arrives DVE is idle:
                        # split the early drains evenly
                        eng = (nc.scalar, nc.vector)[ndrain % 2]
                    else:
                        eng = drain_seq[ndrain % len(drain_seq)]
                    ndrain += 1
                    if eng is nc.scalar:
                        eng.copy(u_hat[:, iblk, h * 2:(h + 1) * 2, :], ps[:])
                    else:
                        eng.tensor_copy(u_hat[:, iblk, h * 2:(h + 1) * 2, :], ps[:])
            if e == 0:
                # rest of the iter-0 GEMM, then its squash + vx broadcasts
                for j in range(18, NIB):
                    nc.tensor.matmul(
                        ps0[:], lhsT=xt_sb[:, j, :], rhs=ws_sb[:, j, :],
                        start=False, stop=(j == NIB - 1), skip_group_check=True)
                _squash(nc, smp, ps0[:], 0.1, vall)
                for bblk in range(NBB):
                    load_vx(bblk, first=True)
            # iter-0 agreement in 18-j sets (every 2nd e-chunk): halves the
            # per-op fixed cost on DVE vs 9-j sets
            if e % 2 == 1:
                for bblk in range(NBB):
                    eng = nc.gpsimd if bblk == (e // 2) % 4 else nc.vector
                    nagr += 1
                    agreement_slice(bblk, (e - 1) * 9, (e + 1) * 9, True, eng)

        # --- iters 1, 2: software-pipelined across bblks and iterations.
        # softmax+cbt feed the PE stage-1 stream; each bblk's iter-2 chain
        # starts as soon as its iter-1 agreement lands. ---
        def softmax_cbt(it, bblk):
            nc.scalar.activation(
                c_sb[:, bblk, :, :], blg[:, bblk, :, :], AF.Exp)
            sm = smp.tile([128, NIB], F32, tag="sm")
            nc.vector.tensor_reduce(
                sm[:], c_sb[:, bblk, :, :], axis=AX.X, op=mybir.AluOpType.add)
            rr = smp.tile([128, NIB], BF, tag="rr")
            with nc.allow_low_precision(reason="softmax denom recip in fp16"):
                nc.vector.reciprocal(rr[:], sm[:])
            cn = cnp.tile([128, NIB, OC], BF, tag="cn")
            nc.vector.tensor_mul(
                cn[:], c_sb[:, bblk, :, :],
                rr[:].unsqueeze(-1).broadcast_to((128, NIB, OC)))
            eng = nc.vector
            cbt = cbp.tile([128, NIB, 80], BF, tag=f"cbt{bblk % 2}")
            eng.tensor_mul(
                cbt[:],
                cn[:].unsqueeze(2).broadcast_to((128, NIB, 8, OC)),
                mk80_sb[:].rearrange("p (b o) -> p b o", o=OC).unsqueeze(1)
                .broadcast_to((128, NIB, 8, OC)))
            ps1 = ps1p.tile([80, F], F32, tag="ps1", name=f"ps1_{it}_{bblk}")
            for j in range(NIB):
                nc.tensor.matmul(
                    ps1[:], lhsT=cbt[:, j, :],
                    rhs=u_hat[:, j, bblk, :],
                    start=(j == 0), stop=(j == NIB - 1))
            return ps1

        def extract_s(ps1):
            mskd = mkp.tile([80, F], BF, tag="mskd")
            nc.vector.tensor_mul(mskd[:], ps1[:], msk_sb[:])
            psv = psvp.tile([8, F], F32, tag="psv")
            nc.tensor.matmul(
                psv[:], lhsT=o80_sb[:], rhs=mskd[:], start=True, stop=True)
            return psv

        def batched_squash(dst):
            """dst[8, NBB, F] = squash(sv_sb) over all 4 b-blocks at once:
            8 ops instead of 32, cutting the latency-bound serial chain."""
            sq4 = st.tile([8, NBB, F], F32, tag="sq4")
            nc.vector.tensor_mul(sq4[:], sv_sb[:], sv_sb[:])
            n24 = st.tile([8, NBB, OC], F32, tag="n24")
            nc.vector.tensor_reduce(
                n24[:], sq4[:].rearrange("p b (o d) -> p b o d", d=OD),
                axis=AX.X, op=mybir.AluOpType.add)
            n14 = st.tile([8, NBB, OC], F32, tag="n14")
            nc.vector.tensor_scalar_add(n14[:], n24[:], 1.0)
            sn4 = st.tile([8, NBB, OC], F32, tag="sn4")
            nc.scalar.sqrt(sn4[:], n24[:])
            t14 = st.tile([8, NBB, OC], F32, tag="t14")
            nc.vector.tensor_mul(t14[:], n14[:], sn4[:])
            r14 = st.tile([8, NBB, OC], F32, tag="r14")
            nc.vector.reciprocal(r14[:], t14[:])
            f14 = st.tile([8, NBB, OC], F32, tag="f14")
            nc.vector.tensor_mul(f14[:], n24[:], r14[:])
            nc.vector.tensor_mul(
                dst[:].rearrange("p b (o d) -> p b o d", d=OD),
                sv_sb[:].rearrange("p b (o d) -> p b o d", d=OD),
                f14[:].unsqueeze(-1).broadcast_to((8, NBB, OC, OD)))

        for it in (1, 2):
            ps1s = [softmax_cbt(it, bb) for bb in range(NBB)]
            for bblk in range(NBB):
                psv = extract_s(ps1s[bblk])
                if it == 1:
                    _squash(nc, smp, psv[:], 1.0, v8[bblk])
                    load_vx(bblk, first=False)
                    agreement_slice(bblk, 0, 27, False, nc.vector)
                    agreement_slice(bblk, 27, 54, False, nc.vector)
                    agreement_slice(bblk, 54, 72, False, nc.gpsimd)
                else:
                    nc.scalar.copy(sv_sb[:, bblk, :], psv[:])
            if it == 2:
                batched_squash(og)
        nc.gpsimd.dma_start(
            out=out_d[:].rearrange("(bb bp) f -> bp bb f", bp=8), in_=og[:])

    if split_waits:
        _split_multiwait(nc)
    return nc


def _host_inputs(x, W):
    """Per-core input maps from full x [256,1152,8] f32, W [1,1152,10,16,8] f32."""
    bf = np.float16
    f8 = mybir.dt.np(BD_DT)
    W0 = np.asarray(W[0], dtype=np.float32)
    # ws[q=(ii,k), j, (o,d)] = W[j*16+ii, o, d, k]
    ws = np.ascontiguousarray(
        W0.reshape(NIB, 16, OC, OD, ID).transpose(1, 4, 0, 2, 3)
        .reshape(128, NIB, F)).astype(bf)
    msk = np.zeros((80, F), dtype=bf)
    for bpp in range(8):
        for o in range(OC):
            msk[bpp * 10 + o, o * OD:(o + 1) * OD] = 1.0
    o80 = np.zeros((80, 8), dtype=bf)
    for p in range(80):
        o80[p, p // 10] = 1.0
    sel = np.zeros((8, 128), dtype=bf)
    for p in range(128):
        sel[p // 16, p] = 1.0
    mk80 = np.zeros((128, 80), dtype=bf)
    for p in range(128):
        mk80[p, (p // 16) * 10:(p // 16) * 10 + 10] = 1.0
    selb = np.zeros((B_LOC, NBB, 128), dtype=bf)
    for bb in range(NBB):
        for p in range(128):
            selb[bb * 8 + p // 16, bb, p] = 1.0

    in_maps = []
    for c in range(N_CORES):
        xc = np.asarray(x[c * B_LOC:(c + 1) * B_LOC], dtype=np.float32)
        # bd[e, q=(ii,k), j, bb, m=(bp,ii')] = x[bb*8+bp, (e*9+j)*16+ii, k] iff ii'==ii
        r = xc.reshape(NBB, 8, 8, 9, 16, ID)          # [bb, bp, e, j, ii, k]
        bd6 = np.zeros((8, 16, ID, 9, NBB, 8, 16), dtype=np.float32)
        for ii in range(16):
            bd6[:, ii, :, :, :, :, ii] = r[:, :, :, :, ii, :].transpose(2, 4, 3, 0, 1)
        bd = np.ascontiguousarray(bd6.reshape(8, 128, 9, NBB, 128)).astype(f8)
        # xt[q=(ii,k), iblk, b] = x[b, iblk*16+ii, k]
        xt = np.ascontiguousarray(
            xc.reshape(B_LOC, NIB, 16, ID).transpose(2, 3, 1, 0)
            .reshape(128, NIB, B_LOC)).astype(bf)
        in_maps.append(
            {"bd": bd, "xt": xt, "ws": ws, "msk": msk, "o80": o80, "sel": sel,
             "mk80": mk80, "selb": selb})
    return in_maps


_NC_CACHE = {}


def kernel(x, W):
    from concourse.bass_utils import run_bass_kernel_spmd

    if "nc" not in _NC_CACHE:
        _NC_CACHE["nc"] = build_program()
    nc = _NC_CACHE["nc"]
    in_maps = _host_inputs(x, W)
    res = run_bass_kernel_spmd(nc, in_maps, core_ids=list(range(N_CORES)))
    out = np.concatenate([r["out"] for r in res.results], axis=0)
    return out.reshape(B_FULL, OC, OD).astype(np.float32)


if __name__ == "__main__":
    nc = build_program()
    print("program built ok")
